# revision 6
# baseline (speedup 1.0000x reference)
"""MoE feed-forward (8 experts, top-2, SwiGLU) on 8 Trainium2 NeuronCores.

Strategy: expert parallelism. Core c owns expert c and computes its expert's
SwiGLU output for all tokens with fp32r (FP22) matmuls, weights resident in
SBUF. Gating (router top-2 softmax) is computed on host in float64 and the
per-expert gating row is shipped as an input; each core scales its expert
output by its gating row, partial outputs are combined with an on-device
ReduceScatter, and the host reassembles the full output.
"""

import os
import sys
import time

sys.path.insert(0, "/opt/trn_rl_repo")

import numpy as np

# ---------------------------------------------------------------------------
# Problem constants (hardcoded per contract)
B, S, D, E, I, TOPK = 2, 2048, 1024, 8, 1408, 2
T = B * S  # 4096 tokens
P = 128
D_T = D // P   # 8 d-tiles
I_T = I // P   # 11 i-tiles
TC = 256       # token chunk (PSUM-bank free dim)
N_CORES = 8

_VERBOSE = bool(int(os.environ.get("KERNEL_VERBOSE", "0")))


def _log(msg):
    if _VERBOSE:
        print(f"[kernel] {msg}", flush=True)


def round_f32r(a: np.ndarray) -> np.ndarray:
    """RNE-round fp32 array to 13 mantissa bits (FP22 / e8m13)."""
    v = np.ascontiguousarray(a, dtype=np.float32).view(np.uint32)
    low = v & np.uint32(0x1FFF)
    base = v & np.uint32(0xFFFFE000)
    lsb = (v >> np.uint32(13)) & np.uint32(1)
    round_up = (low > np.uint32(0x1000)) | ((low == np.uint32(0x1000)) & (lsb == 1))
    out = base + (round_up.astype(np.uint32) << np.uint32(13))
    return out.view(np.float32)


def host_gating(x2d: np.ndarray, gate_w: np.ndarray):
    """Exact router: scores -> top-2 -> softmax. Returns gating [T, E] fp32."""
    scores = x2d.astype(np.float64) @ gate_w.astype(np.float64).T  # [T, E]
    idx = np.argsort(-scores, axis=-1, kind="stable")[:, :TOPK]  # [T, 2]
    top = np.take_along_axis(scores, idx, axis=-1)  # [T, 2] descending
    m = top[:, :1]
    ex = np.exp(top - m)
    probs = ex / ex.sum(axis=-1, keepdims=True)  # [T, 2]
    gating = np.zeros((x2d.shape[0], E), dtype=np.float64)
    np.put_along_axis(gating, idx, probs, axis=-1)
    return gating.astype(np.float32)


# ---------------------------------------------------------------------------
# Bass kernel builder


def build_nc(t_total=T, tc=TC, n_cores=N_CORES):
    import concourse.bass as bass  # noqa: F401
    import concourse.mybir as mybir
    import concourse.tile as tile
    from concourse import bacc

    f32 = mybir.dt.float32
    f32r = mybir.dt.float32r
    n_chunks = t_total // tc

    nc = bacc.Bacc("TRN2", debug=False, num_devices=n_cores)

    xT_d = nc.dram_tensor("xT", [D, t_total], f32r, kind="ExternalInput")
    wgT_d = nc.dram_tensor("wgT", [D, I], f32r, kind="ExternalInput")
    wuT_d = nc.dram_tensor("wuT", [D, I], f32r, kind="ExternalInput")
    wdT_d = nc.dram_tensor("wdT", [I, D], f32r, kind="ExternalInput")
    gcol_d = nc.dram_tensor("gcol", [1, t_total], f32r, kind="ExternalInput")
    ones_d = nc.dram_tensor("ones", [1, P], f32r, kind="ExternalInput")
    yshard_d = nc.dram_tensor("yshard", [D * t_total // n_cores], f32,
                              kind="ExternalOutput")

    xT_r = xT_d.ap().rearrange("(do dp) t -> dp do t", dp=P)
    wgT_r = wgT_d.ap().rearrange("(do dp) i -> dp do i", dp=P)
    wuT_r = wuT_d.ap().rearrange("(do dp) i -> dp do i", dp=P)
    wdT_r = wdT_d.ap().rearrange("(io ip) d -> ip io d", ip=P)

    with tile.TileContext(nc) as tc_ctx:
        tcx = tc_ctx
        with tcx.tile_pool(name="wpool", bufs=1) as wpool, \
             tcx.tile_pool(name="xpool", bufs=2) as xpool, \
             tcx.tile_pool(name="hpool", bufs=2) as hpool, \
             tcx.tile_pool(name="ypool", bufs=2) as ypool, \
             tcx.tile_pool(name="gspool", bufs=3) as gspool, \
             tcx.tile_pool(name="gbpool", bufs=2) as gbpool, \
             tcx.tile_pool(name="psg", bufs=2, space="PSUM") as psg, \
             tcx.tile_pool(name="psu", bufs=2, space="PSUM") as psu, \
             tcx.tile_pool(name="psy", bufs=2, space="PSUM") as psy, \
             tcx.tile_pool(name="psb", bufs=1, space="PSUM") as psb, \
             tcx.tile_pool(name="dram", bufs=1, space="DRAM") as dram:

            # ---- resident weights ----
            wg_sb = wpool.tile([P, D_T, I], f32r)
            wu_sb = wpool.tile([P, D_T, I], f32r)
            wd_sb = wpool.tile([P, I_T, D], f32r)
            for d_o in range(D_T):
                nc.sync.dma_start(wg_sb[:, d_o, :], wgT_r[:, d_o, :])
                nc.sync.dma_start(wu_sb[:, d_o, :], wuT_r[:, d_o, :])
            for i_o in range(I_T):
                nc.sync.dma_start(wd_sb[:, i_o, :], wdT_r[:, i_o, :])

            # gating row + ones column for partition-broadcast matmul
            gcol_sb = wpool.tile([1, t_total], f32r)
            nc.sync.dma_start(gcol_sb[:], gcol_d.ap())
            ones_sb = wpool.tile([1, P], f32r)
            nc.sync.dma_start(ones_sb[:], ones_d.ap())

            partial = dram.tile([D, t_total], f32)
            partial_r = partial.rearrange("(do dp) t -> dp do t", dp=P)
            rs_out = dram.tile([D * t_total // n_cores], f32)

            for ci in range(n_chunks):
                t0 = ci * tc
                xt = xpool.tile([P, D_T, tc], f32r, tag="xt")
                half = D_T // 2
                nc.sync.dma_start(xt[:, :half, :], xT_r[:, :half, t0:t0 + tc])
                nc.sync.dma_start(xt[:, half:, :], xT_r[:, half:, t0:t0 + tc])

                # broadcast gating row to 128 partitions for this chunk
                gb_ps = psb.tile([P, tc], f32, tag="gbps")
                nc.tensor.matmul(gb_ps[:], ones_sb[:], gcol_sb[:, t0:t0 + tc],
                                 start=True, stop=True)
                gb_sb = gbpool.tile([P, tc], f32, tag="gb")
                nc.scalar.copy(out=gb_sb[:], in_=gb_ps[:])

                h = hpool.tile([P, I_T, tc], f32r, tag="h")
                for i_o in range(I_T):
                    pg = psg.tile([P, tc], f32, tag="pg")
                    pu = psu.tile([P, tc], f32, tag="pu")
                    for d_o in range(D_T):
                        nc.tensor.matmul(
                            pg[:], wg_sb[:, d_o, i_o * P:(i_o + 1) * P],
                            xt[:, d_o, :],
                            start=(d_o == 0), stop=(d_o == D_T - 1))
                    for d_o in range(D_T):
                        nc.tensor.matmul(
                            pu[:], wu_sb[:, d_o, i_o * P:(i_o + 1) * P],
                            xt[:, d_o, :],
                            start=(d_o == 0), stop=(d_o == D_T - 1))
                    gs = gspool.tile([P, tc], f32r, tag="gs")
                    nc.scalar.activation(gs[:], pg[:],
                                         mybir.ActivationFunctionType.Silu)
                    nc.vector.tensor_mul(out=h[:, i_o, :], in0=gs[:], in1=pu[:])

                yout = ypool.tile([P, D_T, tc], f32, tag="yout")
                for d_o in range(D_T):
                    py = psy.tile([P, tc], f32, tag="py")
                    for i_o in range(I_T):
                        nc.tensor.matmul(
                            py[:], wd_sb[:, i_o, d_o * P:(d_o + 1) * P],
                            h[:, i_o, :],
                            start=(i_o == 0), stop=(i_o == I_T - 1))
                    nc.vector.tensor_mul(out=yout[:, d_o, :], in0=py[:],
                                         in1=gb_sb[:])
                nc.sync.dma_start(partial_r[:, :half, t0:t0 + tc],
                                  yout[:, :half, :])
                nc.sync.dma_start(partial_r[:, half:, t0:t0 + tc],
                                  yout[:, half:, :])

            nc.gpsimd.collective_compute(
                "ReduceScatter", mybir.AluOpType.add,
                replica_groups=[list(range(n_cores))],
                ins=[partial[:].opt()], outs=[rs_out[:].opt()])

            shard = D * t_total // n_cores
            q = shard // 4
            for k in range(4):
                nc.sync.dma_start(yshard_d.ap()[k * q:(k + 1) * q],
                                  rs_out[k * q:(k + 1) * q])

    nc.compile()
    return nc


def build_nc_routed(cap, tc=TC, n_cores=N_CORES):
    """Compact (routed) variant: each core computes its expert only for the
    `cap` tokens routed to it (host-gathered, feature-major). Output is the
    compact gated expert output, token-major [cap, D]; host scatter-adds."""
    import concourse.mybir as mybir
    import concourse.tile as tile
    from concourse import bacc

    f32 = mybir.dt.float32
    f32r = mybir.dt.float32r
    n_chunks = cap // tc
    assert cap % tc == 0

    nc = bacc.Bacc("TRN2", debug=False, num_devices=n_cores)

    xcT_d = nc.dram_tensor("xcT", [D, cap], f32r, kind="ExternalInput")
    wgT_d = nc.dram_tensor("wgT", [D, I], f32r, kind="ExternalInput")
    wuT_d = nc.dram_tensor("wuT", [D, I], f32r, kind="ExternalInput")
    wdT_d = nc.dram_tensor("wdT", [I, D], f32r, kind="ExternalInput")
    gprob_d = nc.dram_tensor("gprob", [1, cap], f32r, kind="ExternalInput")
    ones_d = nc.dram_tensor("ones", [1, P], f32r, kind="ExternalInput")
    ycomp_d = nc.dram_tensor("ycomp", [cap, D], f32, kind="ExternalOutput")

    xcT_r = xcT_d.ap().rearrange("(do dp) t -> dp do t", dp=P)
    wgT_r = wgT_d.ap().rearrange("(do dp) i -> dp do i", dp=P)
    wuT_r = wuT_d.ap().rearrange("(do dp) i -> dp do i", dp=P)
    wdT_r = wdT_d.ap().rearrange("(io ip) d -> ip io d", ip=P)
    # ycomp viewed as [dp, do, t] for direct writes from [d-major] tiles
    ycomp_r = ycomp_d.ap().rearrange("t (do dp) -> dp do t", dp=P)

    with tile.TileContext(nc) as tcx:
        with tcx.tile_pool(name="wpool", bufs=1) as wpool, \
             tcx.tile_pool(name="xpool", bufs=2) as xpool, \
             tcx.tile_pool(name="hpool", bufs=2) as hpool, \
             tcx.tile_pool(name="ypool", bufs=2) as ypool, \
             tcx.tile_pool(name="gspool", bufs=3) as gspool, \
             tcx.tile_pool(name="gbpool", bufs=2) as gbpool, \
             tcx.tile_pool(name="psg", bufs=2, space="PSUM") as psg, \
             tcx.tile_pool(name="psu", bufs=2, space="PSUM") as psu, \
             tcx.tile_pool(name="psy", bufs=2, space="PSUM") as psy, \
             tcx.tile_pool(name="psb", bufs=1, space="PSUM") as psb:

            wg_sb = wpool.tile([P, D_T, I], f32r)
            wu_sb = wpool.tile([P, D_T, I], f32r)
            wd_sb = wpool.tile([P, I_T, D], f32r)
            for d_o in range(D_T):
                nc.sync.dma_start(wg_sb[:, d_o, :], wgT_r[:, d_o, :])
                nc.sync.dma_start(wu_sb[:, d_o, :], wuT_r[:, d_o, :])
            for i_o in range(I_T):
                nc.sync.dma_start(wd_sb[:, i_o, :], wdT_r[:, i_o, :])

            gprob_sb = wpool.tile([1, cap], f32r)
            nc.sync.dma_start(gprob_sb[:], gprob_d.ap())
            ones_sb = wpool.tile([1, P], f32r)
            nc.sync.dma_start(ones_sb[:], ones_d.ap())

            half = D_T // 2
            for ci in range(n_chunks):
                t0 = ci * tc
                xt = xpool.tile([P, D_T, tc], f32r, tag="xt")
                nc.sync.dma_start(xt[:, :half, :], xcT_r[:, :half, t0:t0 + tc])
                nc.sync.dma_start(xt[:, half:, :], xcT_r[:, half:, t0:t0 + tc])

                gb_ps = psb.tile([P, tc], f32, tag="gbps")
                nc.tensor.matmul(gb_ps[:], ones_sb[:], gprob_sb[:, t0:t0 + tc],
                                 start=True, stop=True)
                gb_sb = gbpool.tile([P, tc], f32, tag="gb")
                nc.scalar.copy(out=gb_sb[:], in_=gb_ps[:])

                h = hpool.tile([P, I_T, tc], f32r, tag="h")
                for i_o in range(I_T):
                    pg = psg.tile([P, tc], f32, tag="pg")
                    pu = psu.tile([P, tc], f32, tag="pu")
                    for d_o in range(D_T):
                        nc.tensor.matmul(
                            pg[:], wg_sb[:, d_o, i_o * P:(i_o + 1) * P],
                            xt[:, d_o, :],
                            start=(d_o == 0), stop=(d_o == D_T - 1))
                    for d_o in range(D_T):
                        nc.tensor.matmul(
                            pu[:], wu_sb[:, d_o, i_o * P:(i_o + 1) * P],
                            xt[:, d_o, :],
                            start=(d_o == 0), stop=(d_o == D_T - 1))
                    gs = gspool.tile([P, tc], f32r, tag="gs")
                    nc.scalar.activation(gs[:], pg[:],
                                         mybir.ActivationFunctionType.Silu)
                    nc.vector.tensor_mul(out=h[:, i_o, :], in0=gs[:], in1=pu[:])

                yout = ypool.tile([P, D_T, tc], f32, tag="yout")
                for d_o in range(D_T):
                    py = psy.tile([P, tc], f32, tag="py")
                    for i_o in range(I_T):
                        nc.tensor.matmul(
                            py[:], wd_sb[:, i_o, d_o * P:(d_o + 1) * P],
                            h[:, i_o, :],
                            start=(i_o == 0), stop=(i_o == I_T - 1))
                    nc.vector.tensor_mul(out=yout[:, d_o, :], in0=py[:],
                                         in1=gb_sb[:])
                for d_o in range(D_T):
                    nc.sync.dma_start(ycomp_r[:, d_o, t0:t0 + tc],
                                      yout[:, d_o, :])

    nc.compile()
    return nc


# ---------------------------------------------------------------------------
# Host-side wrapper

_CACHED = {}


def _get_nc(t_total=T, tc=TC, n_cores=N_CORES):
    key = (t_total, tc, n_cores)
    if key not in _CACHED:
        t0 = time.time()
        _CACHED[key] = build_nc(t_total, tc, n_cores)
        _log(f"built bass program in {time.time() - t0:.1f}s")
    return _CACHED[key]


def make_in_maps(x, gate_w, gate_proj_w, up_proj_w, down_proj_w,
                 t_total=T, n_cores=N_CORES):
    x2d = np.ascontiguousarray(np.asarray(x, dtype=np.float32).reshape(t_total, D))
    xT = round_f32r(x2d.T)  # [D, T]
    gating = host_gating(x2d, np.asarray(gate_w, dtype=np.float32))  # [T, E]
    gating_r = round_f32r(gating.T)  # [E, T]
    in_maps = []
    for c in range(n_cores):
        in_maps.append({
            "xT": xT,
            "wgT": round_f32r(np.asarray(gate_proj_w[c], np.float32).T),
            "wuT": round_f32r(np.asarray(up_proj_w[c], np.float32).T),
            "wdT": round_f32r(np.asarray(down_proj_w[c], np.float32).T),
            "gcol": gating_r[c:c + 1, :],
            "ones": np.ones((1, P), dtype=np.float32),
        })
    return in_maps


def assemble_output(results, t_total=T, n_cores=N_CORES):
    shard = D // n_cores
    yT = np.empty((D, t_total), dtype=np.float32)
    for c in range(n_cores):
        yT[c * shard:(c + 1) * shard, :] = \
            results[c]["yshard"].reshape(shard, t_total)
    return np.ascontiguousarray(yT.T).reshape(B, S, D)


def _get_nc_routed(cap, tc=TC, n_cores=N_CORES):
    key = ("routed", cap, tc, n_cores)
    if key not in _CACHED:
        t0 = time.time()
        _CACHED[key] = build_nc_routed(cap, tc, n_cores)
        _log(f"built routed bass program (cap={cap}) in {time.time() - t0:.1f}s")
    return _CACHED[key]


def _round_up(v, m):
    return (v + m - 1) // m * m


def make_in_maps_routed(x, gate_w, gate_proj_w, up_proj_w, down_proj_w):
    """Returns (in_maps, idx_list, n_list, cap)."""
    from concurrent.futures import ThreadPoolExecutor

    x2d = np.ascontiguousarray(np.asarray(x, np.float32).reshape(T, D))
    x2d_r = round_f32r(x2d)
    gating = host_gating(x2d, np.asarray(gate_w, np.float32))  # [T, E]
    idx_list = [np.nonzero(gating[:, c] > 0)[0].astype(np.int64)
                for c in range(N_CORES)]
    n_list = [len(ix) for ix in idx_list]
    cap = _round_up(max(n_list), TC)

    ones = np.ones((1, P), dtype=np.float32)

    def prep_core(c):
        ix, n_c = idx_list[c], n_list[c]
        xcT = np.zeros((D, cap), dtype=np.float32)
        xcT[:, :n_c] = x2d_r[ix].T
        gprob = np.zeros((1, cap), dtype=np.float32)
        gprob[0, :n_c] = gating[ix, c]
        return {
            "xcT": xcT,
            "wgT": round_f32r(np.asarray(gate_proj_w[c], np.float32).T),
            "wuT": round_f32r(np.asarray(up_proj_w[c], np.float32).T),
            "wdT": round_f32r(np.asarray(down_proj_w[c], np.float32).T),
            "gprob": round_f32r(gprob),
            "ones": ones,
        }

    with ThreadPoolExecutor(N_CORES) as ex:
        in_maps = list(ex.map(prep_core, range(N_CORES)))
    return in_maps, idx_list, n_list, cap


def kernel(x, gate_w, gate_proj_w, up_proj_w, down_proj_w,
           num_experts_per_tok=2, _trace=False, _trace_cores=None):
    from concourse import bass_utils
    assert int(num_experts_per_tok) == TOPK
    mode = os.environ.get("KERNEL_MODE", "routed")

    kwargs = {}
    if _trace:
        try:
            sys.path.insert(0, os.path.dirname(os.path.abspath(__file__)))
            import axon_profile_shim
            axon_profile_shim.install()
        except Exception as exc:  # profiling is best-effort
            _log(f"profile shim unavailable: {exc}")
        kwargs = dict(trace=True,
                      trace_cores=_trace_cores or list(range(N_CORES)))

    if mode == "dense":
        nc = _get_nc()
        in_maps = make_in_maps(x, gate_w, gate_proj_w, up_proj_w, down_proj_w)
        t0 = time.time()
        res = bass_utils.run_bass_kernel_spmd(
            nc, in_maps, core_ids=list(range(N_CORES)), **kwargs)
        _log(f"run_bass_kernel_spmd took {time.time() - t0:.1f}s")
        kernel.last_result = res
        return assemble_output(res.results)

    # routed (default)
    t0 = time.time()
    in_maps, idx_list, n_list, cap = make_in_maps_routed(
        x, gate_w, gate_proj_w, up_proj_w, down_proj_w)
    _log(f"host prep {time.time() - t0:.1f}s (cap={cap}, counts={n_list})")
    nc = _get_nc_routed(cap)
    t0 = time.time()
    res = bass_utils.run_bass_kernel_spmd(
        nc, in_maps, core_ids=list(range(N_CORES)), **kwargs)
    _log(f"run_bass_kernel_spmd took {time.time() - t0:.1f}s")
    kernel.last_result = res
    t0 = time.time()
    y = np.zeros((T, D), dtype=np.float32)
    for c in range(N_CORES):
        yc = res.results[c]["ycomp"]
        y[idx_list[c]] += yc[:n_list[c]]
    _log(f"host combine {time.time() - t0:.1f}s")
    return y.reshape(B, S, D)


kernel.last_result = None


# revision 7
# speedup vs baseline: 25.7236x; 25.7236x over previous
"""MoE feed-forward (8 experts, top-2, SwiGLU) on 8 Trainium2 NeuronCores.

Strategy: expert parallelism. Core c owns expert c and computes its expert's
SwiGLU output for all tokens with fp32r (FP22) matmuls, weights resident in
SBUF. Gating (router top-2 softmax) is computed on host in float64 and the
per-expert gating row is shipped as an input; each core scales its expert
output by its gating row, partial outputs are combined with an on-device
ReduceScatter, and the host reassembles the full output.
"""

import os
import sys
import time

sys.path.insert(0, "/opt/trn_rl_repo")

import numpy as np

# ---------------------------------------------------------------------------
# Problem constants (hardcoded per contract)
B, S, D, E, I, TOPK = 2, 2048, 1024, 8, 1408, 2
T = B * S  # 4096 tokens
P = 128
D_T = D // P   # 8 d-tiles
I_T = I // P   # 11 i-tiles
TC = 256       # token chunk (PSUM-bank free dim)
N_CORES = 8

_VERBOSE = bool(int(os.environ.get("KERNEL_VERBOSE", "0")))


def _log(msg):
    if _VERBOSE:
        print(f"[kernel] {msg}", flush=True)


def round_f32r(a: np.ndarray) -> np.ndarray:
    """RNE-round fp32 array to 13 mantissa bits (FP22 / e8m13)."""
    v = np.ascontiguousarray(a, dtype=np.float32).view(np.uint32)
    low = v & np.uint32(0x1FFF)
    base = v & np.uint32(0xFFFFE000)
    lsb = (v >> np.uint32(13)) & np.uint32(1)
    round_up = (low > np.uint32(0x1000)) | ((low == np.uint32(0x1000)) & (lsb == 1))
    out = base + (round_up.astype(np.uint32) << np.uint32(13))
    return out.view(np.float32)


def host_gating(x2d: np.ndarray, gate_w: np.ndarray):
    """Exact router: scores -> top-2 -> softmax. Returns gating [T, E] fp32."""
    scores = x2d.astype(np.float64) @ gate_w.astype(np.float64).T  # [T, E]
    idx = np.argsort(-scores, axis=-1, kind="stable")[:, :TOPK]  # [T, 2]
    top = np.take_along_axis(scores, idx, axis=-1)  # [T, 2] descending
    m = top[:, :1]
    ex = np.exp(top - m)
    probs = ex / ex.sum(axis=-1, keepdims=True)  # [T, 2]
    gating = np.zeros((x2d.shape[0], E), dtype=np.float64)
    np.put_along_axis(gating, idx, probs, axis=-1)
    return gating.astype(np.float32)


# ---------------------------------------------------------------------------
# Bass kernel builder


def build_nc(t_total=T, tc=TC, n_cores=N_CORES):
    import concourse.bass as bass  # noqa: F401
    import concourse.mybir as mybir
    import concourse.tile as tile
    from concourse import bacc

    f32 = mybir.dt.float32
    f32r = mybir.dt.float32r
    n_chunks = t_total // tc

    nc = bacc.Bacc("TRN2", debug=False, num_devices=n_cores)

    xT_d = nc.dram_tensor("xT", [D, t_total], f32r, kind="ExternalInput")
    wgT_d = nc.dram_tensor("wgT", [D, I], f32r, kind="ExternalInput")
    wuT_d = nc.dram_tensor("wuT", [D, I], f32r, kind="ExternalInput")
    wdT_d = nc.dram_tensor("wdT", [I, D], f32r, kind="ExternalInput")
    gcol_d = nc.dram_tensor("gcol", [1, t_total], f32r, kind="ExternalInput")
    ones_d = nc.dram_tensor("ones", [1, P], f32r, kind="ExternalInput")
    yshard_d = nc.dram_tensor("yshard", [D * t_total // n_cores], f32,
                              kind="ExternalOutput")

    xT_r = xT_d.ap().rearrange("(do dp) t -> dp do t", dp=P)
    wgT_r = wgT_d.ap().rearrange("(do dp) i -> dp do i", dp=P)
    wuT_r = wuT_d.ap().rearrange("(do dp) i -> dp do i", dp=P)
    wdT_r = wdT_d.ap().rearrange("(io ip) d -> ip io d", ip=P)

    with tile.TileContext(nc) as tc_ctx:
        tcx = tc_ctx
        with tcx.tile_pool(name="wpool", bufs=1) as wpool, \
             tcx.tile_pool(name="xpool", bufs=2) as xpool, \
             tcx.tile_pool(name="hpool", bufs=2) as hpool, \
             tcx.tile_pool(name="ypool", bufs=2) as ypool, \
             tcx.tile_pool(name="gspool", bufs=3) as gspool, \
             tcx.tile_pool(name="gbpool", bufs=2) as gbpool, \
             tcx.tile_pool(name="psg", bufs=2, space="PSUM") as psg, \
             tcx.tile_pool(name="psu", bufs=2, space="PSUM") as psu, \
             tcx.tile_pool(name="psy", bufs=2, space="PSUM") as psy, \
             tcx.tile_pool(name="psb", bufs=1, space="PSUM") as psb, \
             tcx.tile_pool(name="dram", bufs=1, space="DRAM") as dram:

            # ---- resident weights ----
            wg_sb = wpool.tile([P, D_T, I], f32r)
            wu_sb = wpool.tile([P, D_T, I], f32r)
            wd_sb = wpool.tile([P, I_T, D], f32r)
            for d_o in range(D_T):
                nc.sync.dma_start(wg_sb[:, d_o, :], wgT_r[:, d_o, :])
                nc.sync.dma_start(wu_sb[:, d_o, :], wuT_r[:, d_o, :])
            for i_o in range(I_T):
                nc.sync.dma_start(wd_sb[:, i_o, :], wdT_r[:, i_o, :])

            # gating row + ones column for partition-broadcast matmul
            gcol_sb = wpool.tile([1, t_total], f32r)
            nc.sync.dma_start(gcol_sb[:], gcol_d.ap())
            ones_sb = wpool.tile([1, P], f32r)
            nc.sync.dma_start(ones_sb[:], ones_d.ap())

            partial = dram.tile([D, t_total], f32)
            partial_r = partial.rearrange("(do dp) t -> dp do t", dp=P)
            rs_out = dram.tile([D * t_total // n_cores], f32)

            for ci in range(n_chunks):
                t0 = ci * tc
                xt = xpool.tile([P, D_T, tc], f32r, tag="xt")
                half = D_T // 2
                nc.sync.dma_start(xt[:, :half, :], xT_r[:, :half, t0:t0 + tc])
                nc.sync.dma_start(xt[:, half:, :], xT_r[:, half:, t0:t0 + tc])

                # broadcast gating row to 128 partitions for this chunk
                gb_ps = psb.tile([P, tc], f32, tag="gbps")
                nc.tensor.matmul(gb_ps[:], ones_sb[:], gcol_sb[:, t0:t0 + tc],
                                 start=True, stop=True)
                gb_sb = gbpool.tile([P, tc], f32, tag="gb")
                nc.scalar.copy(out=gb_sb[:], in_=gb_ps[:])

                h = hpool.tile([P, I_T, tc], f32r, tag="h")
                for i_o in range(I_T):
                    pg = psg.tile([P, tc], f32, tag="pg")
                    pu = psu.tile([P, tc], f32, tag="pu")
                    for d_o in range(D_T):
                        nc.tensor.matmul(
                            pg[:], wg_sb[:, d_o, i_o * P:(i_o + 1) * P],
                            xt[:, d_o, :],
                            start=(d_o == 0), stop=(d_o == D_T - 1))
                    for d_o in range(D_T):
                        nc.tensor.matmul(
                            pu[:], wu_sb[:, d_o, i_o * P:(i_o + 1) * P],
                            xt[:, d_o, :],
                            start=(d_o == 0), stop=(d_o == D_T - 1))
                    gs = gspool.tile([P, tc], f32r, tag="gs")
                    nc.scalar.activation(gs[:], pg[:],
                                         mybir.ActivationFunctionType.Silu)
                    nc.vector.tensor_mul(out=h[:, i_o, :], in0=gs[:], in1=pu[:])

                yout = ypool.tile([P, D_T, tc], f32, tag="yout")
                for d_o in range(D_T):
                    py = psy.tile([P, tc], f32, tag="py")
                    for i_o in range(I_T):
                        nc.tensor.matmul(
                            py[:], wd_sb[:, i_o, d_o * P:(d_o + 1) * P],
                            h[:, i_o, :],
                            start=(i_o == 0), stop=(i_o == I_T - 1))
                    nc.vector.tensor_mul(out=yout[:, d_o, :], in0=py[:],
                                         in1=gb_sb[:])
                nc.sync.dma_start(partial_r[:, :half, t0:t0 + tc],
                                  yout[:, :half, :])
                nc.sync.dma_start(partial_r[:, half:, t0:t0 + tc],
                                  yout[:, half:, :])

            nc.gpsimd.collective_compute(
                "ReduceScatter", mybir.AluOpType.add,
                replica_groups=[list(range(n_cores))],
                ins=[partial[:].opt()], outs=[rs_out[:].opt()])

            shard = D * t_total // n_cores
            q = shard // 4
            for k in range(4):
                nc.sync.dma_start(yshard_d.ap()[k * q:(k + 1) * q],
                                  rs_out[k * q:(k + 1) * q])

    nc.compile()
    return nc


def build_nc_routed(cap, tc=TC, n_cores=N_CORES):
    """Compact (routed) variant: each core computes its expert only for the
    `cap` tokens routed to it (host-gathered, feature-major). Output is the
    compact gated expert output, token-major [cap, D]; host scatter-adds."""
    import concourse.mybir as mybir
    import concourse.tile as tile
    from concourse import bacc

    f32 = mybir.dt.float32
    f32r = mybir.dt.float32r
    n_chunks = cap // tc
    assert cap % tc == 0

    nc = bacc.Bacc("TRN2", debug=False, num_devices=n_cores)

    xcT_d = nc.dram_tensor("xcT", [D, cap], f32r, kind="ExternalInput")
    wgT_d = nc.dram_tensor("wgT", [D, I], f32r, kind="ExternalInput")
    wuT_d = nc.dram_tensor("wuT", [D, I], f32r, kind="ExternalInput")
    wdT_d = nc.dram_tensor("wdT", [I, D], f32r, kind="ExternalInput")
    gprob_d = nc.dram_tensor("gprob", [1, cap], f32r, kind="ExternalInput")
    ones_d = nc.dram_tensor("ones", [1, P], f32r, kind="ExternalInput")
    ycomp_d = nc.dram_tensor("ycomp", [D, cap], f32, kind="ExternalOutput")

    xcT_r = xcT_d.ap().rearrange("(do dp) t -> dp do t", dp=P)
    wgT_r = wgT_d.ap().rearrange("(do dp) i -> dp do i", dp=P)
    wuT_r = wuT_d.ap().rearrange("(do dp) i -> dp do i", dp=P)
    wdT_r = wdT_d.ap().rearrange("(io ip) d -> ip io d", ip=P)
    # ycomp viewed as [dp, do, t]; D-major so each partition writes
    # contiguous `tc`-element runs (per-partition contiguity = DMA speed)
    ycomp_r = ycomp_d.ap().rearrange("(do dp) t -> dp do t", dp=P)

    with tile.TileContext(nc) as tcx:
        with tcx.tile_pool(name="wpool", bufs=1) as wpool, \
             tcx.tile_pool(name="xpool", bufs=2) as xpool, \
             tcx.tile_pool(name="hpool", bufs=2) as hpool, \
             tcx.tile_pool(name="ypool", bufs=2) as ypool, \
             tcx.tile_pool(name="gspool", bufs=3) as gspool, \
             tcx.tile_pool(name="gbpool", bufs=2) as gbpool, \
             tcx.tile_pool(name="psg", bufs=2, space="PSUM") as psg, \
             tcx.tile_pool(name="psu", bufs=2, space="PSUM") as psu, \
             tcx.tile_pool(name="psy", bufs=2, space="PSUM") as psy, \
             tcx.tile_pool(name="psb", bufs=1, space="PSUM") as psb:

            wg_sb = wpool.tile([P, D_T, I], f32r)
            wu_sb = wpool.tile([P, D_T, I], f32r)
            wd_sb = wpool.tile([P, I_T, D], f32r)
            for d_o in range(D_T):
                nc.sync.dma_start(wg_sb[:, d_o, :], wgT_r[:, d_o, :])
                nc.sync.dma_start(wu_sb[:, d_o, :], wuT_r[:, d_o, :])
            for i_o in range(I_T):
                nc.sync.dma_start(wd_sb[:, i_o, :], wdT_r[:, i_o, :])

            gprob_sb = wpool.tile([1, cap], f32r)
            nc.sync.dma_start(gprob_sb[:], gprob_d.ap())
            ones_sb = wpool.tile([1, P], f32r)
            nc.sync.dma_start(ones_sb[:], ones_d.ap())

            half = D_T // 2
            for ci in range(n_chunks):
                t0 = ci * tc
                xt = xpool.tile([P, D_T, tc], f32r, tag="xt")
                nc.sync.dma_start(xt[:, :half, :], xcT_r[:, :half, t0:t0 + tc])
                nc.sync.dma_start(xt[:, half:, :], xcT_r[:, half:, t0:t0 + tc])

                gb_ps = psb.tile([P, tc], f32, tag="gbps")
                nc.tensor.matmul(gb_ps[:], ones_sb[:], gprob_sb[:, t0:t0 + tc],
                                 start=True, stop=True)
                gb_sb = gbpool.tile([P, tc], f32, tag="gb")
                nc.scalar.copy(out=gb_sb[:], in_=gb_ps[:])

                h = hpool.tile([P, I_T, tc], f32r, tag="h")
                for i_o in range(I_T):
                    pg = psg.tile([P, tc], f32, tag="pg")
                    pu = psu.tile([P, tc], f32, tag="pu")
                    for d_o in range(D_T):
                        nc.tensor.matmul(
                            pg[:], wg_sb[:, d_o, i_o * P:(i_o + 1) * P],
                            xt[:, d_o, :],
                            start=(d_o == 0), stop=(d_o == D_T - 1))
                    for d_o in range(D_T):
                        nc.tensor.matmul(
                            pu[:], wu_sb[:, d_o, i_o * P:(i_o + 1) * P],
                            xt[:, d_o, :],
                            start=(d_o == 0), stop=(d_o == D_T - 1))
                    gs = gspool.tile([P, tc], f32r, tag="gs")
                    nc.scalar.activation(gs[:], pg[:],
                                         mybir.ActivationFunctionType.Silu)
                    nc.vector.tensor_mul(out=h[:, i_o, :], in0=gs[:], in1=pu[:])

                yout = ypool.tile([P, D_T, tc], f32, tag="yout")
                for d_o in range(D_T):
                    py = psy.tile([P, tc], f32, tag="py")
                    for i_o in range(I_T):
                        nc.tensor.matmul(
                            py[:], wd_sb[:, i_o, d_o * P:(d_o + 1) * P],
                            h[:, i_o, :],
                            start=(i_o == 0), stop=(i_o == I_T - 1))
                    nc.vector.tensor_mul(out=yout[:, d_o, :], in0=py[:],
                                         in1=gb_sb[:])
                nc.sync.dma_start(ycomp_r[:, :half, t0:t0 + tc],
                                  yout[:, :half, :])
                nc.sync.dma_start(ycomp_r[:, half:, t0:t0 + tc],
                                  yout[:, half:, :])

    nc.compile()
    return nc


# ---------------------------------------------------------------------------
# Host-side wrapper

_CACHED = {}


def _get_nc(t_total=T, tc=TC, n_cores=N_CORES):
    key = (t_total, tc, n_cores)
    if key not in _CACHED:
        t0 = time.time()
        _CACHED[key] = build_nc(t_total, tc, n_cores)
        _log(f"built bass program in {time.time() - t0:.1f}s")
    return _CACHED[key]


def make_in_maps(x, gate_w, gate_proj_w, up_proj_w, down_proj_w,
                 t_total=T, n_cores=N_CORES):
    x2d = np.ascontiguousarray(np.asarray(x, dtype=np.float32).reshape(t_total, D))
    xT = round_f32r(x2d.T)  # [D, T]
    gating = host_gating(x2d, np.asarray(gate_w, dtype=np.float32))  # [T, E]
    gating_r = round_f32r(gating.T)  # [E, T]
    in_maps = []
    for c in range(n_cores):
        in_maps.append({
            "xT": xT,
            "wgT": round_f32r(np.asarray(gate_proj_w[c], np.float32).T),
            "wuT": round_f32r(np.asarray(up_proj_w[c], np.float32).T),
            "wdT": round_f32r(np.asarray(down_proj_w[c], np.float32).T),
            "gcol": gating_r[c:c + 1, :],
            "ones": np.ones((1, P), dtype=np.float32),
        })
    return in_maps


def assemble_output(results, t_total=T, n_cores=N_CORES):
    shard = D // n_cores
    yT = np.empty((D, t_total), dtype=np.float32)
    for c in range(n_cores):
        yT[c * shard:(c + 1) * shard, :] = \
            results[c]["yshard"].reshape(shard, t_total)
    return np.ascontiguousarray(yT.T).reshape(B, S, D)


def _get_nc_routed(cap, tc=TC, n_cores=N_CORES):
    key = ("routed", cap, tc, n_cores)
    if key not in _CACHED:
        t0 = time.time()
        _CACHED[key] = build_nc_routed(cap, tc, n_cores)
        _log(f"built routed bass program (cap={cap}) in {time.time() - t0:.1f}s")
    return _CACHED[key]


def _round_up(v, m):
    return (v + m - 1) // m * m


def make_in_maps_routed(x, gate_w, gate_proj_w, up_proj_w, down_proj_w):
    """Returns (in_maps, idx_list, n_list, cap)."""
    from concurrent.futures import ThreadPoolExecutor

    x2d = np.ascontiguousarray(np.asarray(x, np.float32).reshape(T, D))
    x2d_r = round_f32r(x2d)
    gating = host_gating(x2d, np.asarray(gate_w, np.float32))  # [T, E]
    idx_list = [np.nonzero(gating[:, c] > 0)[0].astype(np.int64)
                for c in range(N_CORES)]
    n_list = [len(ix) for ix in idx_list]
    cap = _round_up(max(n_list), TC)

    ones = np.ones((1, P), dtype=np.float32)

    def prep_core(c):
        ix, n_c = idx_list[c], n_list[c]
        xcT = np.zeros((D, cap), dtype=np.float32)
        xcT[:, :n_c] = x2d_r[ix].T
        gprob = np.zeros((1, cap), dtype=np.float32)
        gprob[0, :n_c] = gating[ix, c]
        return {
            "xcT": xcT,
            "wgT": round_f32r(np.asarray(gate_proj_w[c], np.float32).T),
            "wuT": round_f32r(np.asarray(up_proj_w[c], np.float32).T),
            "wdT": round_f32r(np.asarray(down_proj_w[c], np.float32).T),
            "gprob": round_f32r(gprob),
            "ones": ones,
        }

    with ThreadPoolExecutor(N_CORES) as ex:
        in_maps = list(ex.map(prep_core, range(N_CORES)))
    return in_maps, idx_list, n_list, cap


def kernel(x, gate_w, gate_proj_w, up_proj_w, down_proj_w,
           num_experts_per_tok=2, _trace=False, _trace_cores=None):
    from concourse import bass_utils
    assert int(num_experts_per_tok) == TOPK
    mode = os.environ.get("KERNEL_MODE", "routed")

    kwargs = {}
    if _trace:
        try:
            sys.path.insert(0, os.path.dirname(os.path.abspath(__file__)))
            import axon_profile_shim
            axon_profile_shim.install()
        except Exception as exc:  # profiling is best-effort
            _log(f"profile shim unavailable: {exc}")
        kwargs = dict(trace=True,
                      trace_cores=_trace_cores or list(range(N_CORES)))

    if mode == "dense":
        nc = _get_nc()
        in_maps = make_in_maps(x, gate_w, gate_proj_w, up_proj_w, down_proj_w)
        t0 = time.time()
        res = bass_utils.run_bass_kernel_spmd(
            nc, in_maps, core_ids=list(range(N_CORES)), **kwargs)
        _log(f"run_bass_kernel_spmd took {time.time() - t0:.1f}s")
        kernel.last_result = res
        return assemble_output(res.results)

    # routed (default)
    t0 = time.time()
    in_maps, idx_list, n_list, cap = make_in_maps_routed(
        x, gate_w, gate_proj_w, up_proj_w, down_proj_w)
    _log(f"host prep {time.time() - t0:.1f}s (cap={cap}, counts={n_list})")
    nc = _get_nc_routed(cap)
    t0 = time.time()
    res = bass_utils.run_bass_kernel_spmd(
        nc, in_maps, core_ids=list(range(N_CORES)), **kwargs)
    _log(f"run_bass_kernel_spmd took {time.time() - t0:.1f}s")
    kernel.last_result = res
    t0 = time.time()
    y = np.zeros((T, D), dtype=np.float32)
    for c in range(N_CORES):
        yc = res.results[c]["ycomp"]  # [D, cap]
        y[idx_list[c]] += np.ascontiguousarray(yc[:, :n_list[c]].T)
    _log(f"host combine {time.time() - t0:.1f}s")
    return y.reshape(B, S, D)


kernel.last_result = None


# revision 9
# speedup vs baseline: 30.5870x; 1.1891x over previous
"""MoE feed-forward (8 experts, top-2, SwiGLU) on 8 Trainium2 NeuronCores.

Strategy: expert parallelism. Core c owns expert c and computes its expert's
SwiGLU output for all tokens with fp32r (FP22) matmuls, weights resident in
SBUF. Gating (router top-2 softmax) is computed on host in float64 and the
per-expert gating row is shipped as an input; each core scales its expert
output by its gating row, partial outputs are combined with an on-device
ReduceScatter, and the host reassembles the full output.
"""

import os
import sys
import time

sys.path.insert(0, "/opt/trn_rl_repo")

import numpy as np

# ---------------------------------------------------------------------------
# Problem constants (hardcoded per contract)
B, S, D, E, I, TOPK = 2, 2048, 1024, 8, 1408, 2
T = B * S  # 4096 tokens
P = 128
D_T = D // P   # 8 d-tiles
I_T = I // P   # 11 i-tiles
TC = 256       # token chunk (PSUM-bank free dim)
N_CORES = 8

_VERBOSE = bool(int(os.environ.get("KERNEL_VERBOSE", "0")))


def _log(msg):
    if _VERBOSE:
        print(f"[kernel] {msg}", flush=True)


def round_f32r(a: np.ndarray) -> np.ndarray:
    """RNE-round fp32 array to 13 mantissa bits (FP22 / e8m13)."""
    v = np.ascontiguousarray(a, dtype=np.float32).view(np.uint32)
    low = v & np.uint32(0x1FFF)
    base = v & np.uint32(0xFFFFE000)
    lsb = (v >> np.uint32(13)) & np.uint32(1)
    round_up = (low > np.uint32(0x1000)) | ((low == np.uint32(0x1000)) & (lsb == 1))
    out = base + (round_up.astype(np.uint32) << np.uint32(13))
    return out.view(np.float32)


def host_gating(x2d: np.ndarray, gate_w: np.ndarray):
    """Exact router: scores -> top-2 -> softmax. Returns gating [T, E] fp32."""
    scores = x2d.astype(np.float64) @ gate_w.astype(np.float64).T  # [T, E]
    idx = np.argsort(-scores, axis=-1, kind="stable")[:, :TOPK]  # [T, 2]
    top = np.take_along_axis(scores, idx, axis=-1)  # [T, 2] descending
    m = top[:, :1]
    ex = np.exp(top - m)
    probs = ex / ex.sum(axis=-1, keepdims=True)  # [T, 2]
    gating = np.zeros((x2d.shape[0], E), dtype=np.float64)
    np.put_along_axis(gating, idx, probs, axis=-1)
    return gating.astype(np.float32)


# ---------------------------------------------------------------------------
# Bass kernel builder


def build_nc(t_total=T, tc=TC, n_cores=N_CORES):
    import concourse.bass as bass  # noqa: F401
    import concourse.mybir as mybir
    import concourse.tile as tile
    from concourse import bacc

    f32 = mybir.dt.float32
    f32r = mybir.dt.float32r
    n_chunks = t_total // tc

    nc = bacc.Bacc("TRN2", debug=False, num_devices=n_cores)

    xT_d = nc.dram_tensor("xT", [D, t_total], f32r, kind="ExternalInput")
    wgT_d = nc.dram_tensor("wgT", [D, I], f32r, kind="ExternalInput")
    wuT_d = nc.dram_tensor("wuT", [D, I], f32r, kind="ExternalInput")
    wdT_d = nc.dram_tensor("wdT", [I, D], f32r, kind="ExternalInput")
    gcol_d = nc.dram_tensor("gcol", [1, t_total], f32r, kind="ExternalInput")
    ones_d = nc.dram_tensor("ones", [1, P], f32r, kind="ExternalInput")
    yshard_d = nc.dram_tensor("yshard", [D * t_total // n_cores], f32,
                              kind="ExternalOutput")

    xT_r = xT_d.ap().rearrange("(do dp) t -> dp do t", dp=P)
    wgT_r = wgT_d.ap().rearrange("(do dp) i -> dp do i", dp=P)
    wuT_r = wuT_d.ap().rearrange("(do dp) i -> dp do i", dp=P)
    wdT_r = wdT_d.ap().rearrange("(io ip) d -> ip io d", ip=P)

    with tile.TileContext(nc) as tc_ctx:
        tcx = tc_ctx
        with tcx.tile_pool(name="wpool", bufs=1) as wpool, \
             tcx.tile_pool(name="xpool", bufs=2) as xpool, \
             tcx.tile_pool(name="hpool", bufs=2) as hpool, \
             tcx.tile_pool(name="ypool", bufs=2) as ypool, \
             tcx.tile_pool(name="gspool", bufs=3) as gspool, \
             tcx.tile_pool(name="gbpool", bufs=2) as gbpool, \
             tcx.tile_pool(name="psg", bufs=2, space="PSUM") as psg, \
             tcx.tile_pool(name="psu", bufs=2, space="PSUM") as psu, \
             tcx.tile_pool(name="psy", bufs=2, space="PSUM") as psy, \
             tcx.tile_pool(name="psb", bufs=1, space="PSUM") as psb, \
             tcx.tile_pool(name="dram", bufs=1, space="DRAM") as dram:

            # ---- resident weights ----
            wg_sb = wpool.tile([P, D_T, I], f32r)
            wu_sb = wpool.tile([P, D_T, I], f32r)
            wd_sb = wpool.tile([P, I_T, D], f32r)
            for d_o in range(D_T):
                nc.sync.dma_start(wg_sb[:, d_o, :], wgT_r[:, d_o, :])
                nc.sync.dma_start(wu_sb[:, d_o, :], wuT_r[:, d_o, :])
            for i_o in range(I_T):
                nc.sync.dma_start(wd_sb[:, i_o, :], wdT_r[:, i_o, :])

            # gating row + ones column for partition-broadcast matmul
            gcol_sb = wpool.tile([1, t_total], f32r)
            nc.sync.dma_start(gcol_sb[:], gcol_d.ap())
            ones_sb = wpool.tile([1, P], f32r)
            nc.sync.dma_start(ones_sb[:], ones_d.ap())

            partial = dram.tile([D, t_total], f32)
            partial_r = partial.rearrange("(do dp) t -> dp do t", dp=P)
            rs_out = dram.tile([D * t_total // n_cores], f32)

            for ci in range(n_chunks):
                t0 = ci * tc
                xt = xpool.tile([P, D_T, tc], f32r, tag="xt")
                half = D_T // 2
                nc.sync.dma_start(xt[:, :half, :], xT_r[:, :half, t0:t0 + tc])
                nc.sync.dma_start(xt[:, half:, :], xT_r[:, half:, t0:t0 + tc])

                # broadcast gating row to 128 partitions for this chunk
                gb_ps = psb.tile([P, tc], f32, tag="gbps")
                nc.tensor.matmul(gb_ps[:], ones_sb[:], gcol_sb[:, t0:t0 + tc],
                                 start=True, stop=True)
                gb_sb = gbpool.tile([P, tc], f32, tag="gb")
                nc.scalar.copy(out=gb_sb[:], in_=gb_ps[:])

                h = hpool.tile([P, I_T, tc], f32r, tag="h")
                for i_o in range(I_T):
                    pg = psg.tile([P, tc], f32, tag="pg")
                    pu = psu.tile([P, tc], f32, tag="pu")
                    for d_o in range(D_T):
                        nc.tensor.matmul(
                            pg[:], wg_sb[:, d_o, i_o * P:(i_o + 1) * P],
                            xt[:, d_o, :],
                            start=(d_o == 0), stop=(d_o == D_T - 1))
                    for d_o in range(D_T):
                        nc.tensor.matmul(
                            pu[:], wu_sb[:, d_o, i_o * P:(i_o + 1) * P],
                            xt[:, d_o, :],
                            start=(d_o == 0), stop=(d_o == D_T - 1))
                    gs = gspool.tile([P, tc], f32r, tag="gs")
                    nc.scalar.activation(gs[:], pg[:],
                                         mybir.ActivationFunctionType.Silu)
                    nc.vector.tensor_mul(out=h[:, i_o, :], in0=gs[:], in1=pu[:])

                yout = ypool.tile([P, D_T, tc], f32, tag="yout")
                for d_o in range(D_T):
                    py = psy.tile([P, tc], f32, tag="py")
                    for i_o in range(I_T):
                        nc.tensor.matmul(
                            py[:], wd_sb[:, i_o, d_o * P:(d_o + 1) * P],
                            h[:, i_o, :],
                            start=(i_o == 0), stop=(i_o == I_T - 1))
                    nc.vector.tensor_mul(out=yout[:, d_o, :], in0=py[:],
                                         in1=gb_sb[:])
                nc.sync.dma_start(partial_r[:, :half, t0:t0 + tc],
                                  yout[:, :half, :])
                nc.sync.dma_start(partial_r[:, half:, t0:t0 + tc],
                                  yout[:, half:, :])

            nc.gpsimd.collective_compute(
                "ReduceScatter", mybir.AluOpType.add,
                replica_groups=[list(range(n_cores))],
                ins=[partial[:].opt()], outs=[rs_out[:].opt()])

            shard = D * t_total // n_cores
            q = shard // 4
            for k in range(4):
                nc.sync.dma_start(yshard_d.ap()[k * q:(k + 1) * q],
                                  rs_out[k * q:(k + 1) * q])

    nc.compile()
    return nc


def build_nc_routed(cap, tc=TC, n_cores=N_CORES):
    """Compact (routed) variant: each core computes its expert only for the
    `cap` tokens routed to it (host-gathered, feature-major). Output is the
    compact gated expert output, token-major [cap, D]; host scatter-adds."""
    import concourse.mybir as mybir
    import concourse.tile as tile
    from concourse import bacc

    f32 = mybir.dt.float32
    f32r = mybir.dt.float32r
    n_chunks = cap // tc
    assert cap % tc == 0

    nc = bacc.Bacc("TRN2", debug=False, num_devices=n_cores)

    xcT_d = nc.dram_tensor("xcT", [D, cap], f32r, kind="ExternalInput")
    wgT_d = nc.dram_tensor("wgT", [D, I], f32r, kind="ExternalInput")
    wuT_d = nc.dram_tensor("wuT", [D, I], f32r, kind="ExternalInput")
    wdT_d = nc.dram_tensor("wdT", [I, D], f32r, kind="ExternalInput")
    gprob_d = nc.dram_tensor("gprob", [1, cap], f32r, kind="ExternalInput")
    ones_d = nc.dram_tensor("ones", [1, P], f32r, kind="ExternalInput")
    ycomp_d = nc.dram_tensor("ycomp", [D, cap], f32, kind="ExternalOutput")

    xcT_r = xcT_d.ap().rearrange("(do dp) t -> dp do t", dp=P)
    wgT_r = wgT_d.ap().rearrange("(do dp) i -> dp do i", dp=P)
    wuT_r = wuT_d.ap().rearrange("(do dp) i -> dp do i", dp=P)
    wdT_r = wdT_d.ap().rearrange("(io ip) d -> ip io d", ip=P)
    # ycomp viewed as [dp, do, t]; D-major so each partition writes
    # contiguous `tc`-element runs (per-partition contiguity = DMA speed)
    ycomp_r = ycomp_d.ap().rearrange("(do dp) t -> dp do t", dp=P)

    with tile.TileContext(nc) as tcx:
        with tcx.tile_pool(name="wpool", bufs=1) as wpool, \
             tcx.tile_pool(name="xpool", bufs=2) as xpool, \
             tcx.tile_pool(name="hpool", bufs=2) as hpool, \
             tcx.tile_pool(name="ypool", bufs=2) as ypool, \
             tcx.tile_pool(name="gspool", bufs=3) as gspool, \
             tcx.tile_pool(name="gbpool", bufs=2) as gbpool, \
             tcx.tile_pool(name="psg", bufs=2, space="PSUM") as psg, \
             tcx.tile_pool(name="psu", bufs=2, space="PSUM") as psu, \
             tcx.tile_pool(name="psy", bufs=2, space="PSUM") as psy, \
             tcx.tile_pool(name="psb", bufs=1, space="PSUM") as psb:

            wg_sb = wpool.tile([P, D_T, I], f32r)
            wu_sb = wpool.tile([P, D_T, I], f32r)
            wd_sb = wpool.tile([P, I_T, D], f32r)
            half = D_T // 2
            ihalf = 6 * P  # i-tile boundary: first 6 i-tiles, then the rest

            def load_xt(ci, n_split=2):
                t0 = ci * tc
                xt = xpool.tile([P, D_T, tc], f32r, tag="xt", name=f"xt{ci}")
                step = D_T // n_split
                for s in range(0, D_T, step):
                    nc.sync.dma_start(xt[:, s:s + step, :],
                                      xcT_r[:, s:s + step, t0:t0 + tc])
                return xt

            # Issue order tuned for time-to-first-matmul: tiny tensors and
            # gate weights first (first-half i-tiles), then chunk-0
            # activations; up/down weights stream behind the first matmuls.
            gprob_sb = wpool.tile([1, cap], f32r)
            nc.sync.dma_start(gprob_sb[:], gprob_d.ap())
            ones_sb = wpool.tile([1, P], f32r)
            nc.sync.dma_start(ones_sb[:], ones_d.ap())
            for d_o in range(D_T):
                nc.sync.dma_start(wg_sb[:, d_o, :ihalf], wgT_r[:, d_o, :ihalf])
            xt_pre = {0: load_xt(0, n_split=4)}
            for d_o in range(D_T):
                nc.sync.dma_start(wu_sb[:, d_o, :ihalf], wuT_r[:, d_o, :ihalf])
            for d_o in range(D_T):
                nc.sync.dma_start(wg_sb[:, d_o, ihalf:], wgT_r[:, d_o, ihalf:])
            if n_chunks > 1:
                xt_pre[1] = load_xt(1)
            for d_o in range(D_T):
                nc.sync.dma_start(wu_sb[:, d_o, ihalf:], wuT_r[:, d_o, ihalf:])
            for i_o in range(I_T):
                nc.sync.dma_start(wd_sb[:, i_o, :], wdT_r[:, i_o, :])

            for ci in range(n_chunks):
                t0 = ci * tc
                xt = xt_pre[ci] if ci in xt_pre else load_xt(ci)

                gb_ps = psb.tile([P, tc], f32, tag="gbps")
                nc.tensor.matmul(gb_ps[:], ones_sb[:], gprob_sb[:, t0:t0 + tc],
                                 start=True, stop=True)
                gb_sb = gbpool.tile([P, tc], f32, tag="gb")
                nc.scalar.copy(out=gb_sb[:], in_=gb_ps[:])

                h = hpool.tile([P, I_T, tc], f32r, tag="h")
                for i_o in range(I_T):
                    pg = psg.tile([P, tc], f32, tag="pg")
                    pu = psu.tile([P, tc], f32, tag="pu")
                    for d_o in range(D_T):
                        nc.tensor.matmul(
                            pg[:], wg_sb[:, d_o, i_o * P:(i_o + 1) * P],
                            xt[:, d_o, :],
                            start=(d_o == 0), stop=(d_o == D_T - 1))
                    for d_o in range(D_T):
                        nc.tensor.matmul(
                            pu[:], wu_sb[:, d_o, i_o * P:(i_o + 1) * P],
                            xt[:, d_o, :],
                            start=(d_o == 0), stop=(d_o == D_T - 1))
                    gs = gspool.tile([P, tc], f32r, tag="gs")
                    nc.scalar.activation(gs[:], pg[:],
                                         mybir.ActivationFunctionType.Silu)
                    nc.vector.tensor_mul(out=h[:, i_o, :], in0=gs[:], in1=pu[:])

                yout = ypool.tile([P, D_T, tc], f32, tag="yout")
                for d_o in range(D_T):
                    py = psy.tile([P, tc], f32, tag="py")
                    for i_o in range(I_T):
                        nc.tensor.matmul(
                            py[:], wd_sb[:, i_o, d_o * P:(d_o + 1) * P],
                            h[:, i_o, :],
                            start=(i_o == 0), stop=(i_o == I_T - 1))
                    nc.vector.tensor_mul(out=yout[:, d_o, :], in0=py[:],
                                         in1=gb_sb[:])
                nc.sync.dma_start(ycomp_r[:, :half, t0:t0 + tc],
                                  yout[:, :half, :])
                nc.sync.dma_start(ycomp_r[:, half:, t0:t0 + tc],
                                  yout[:, half:, :])

    nc.compile()
    return nc


# ---------------------------------------------------------------------------
# Host-side wrapper

_CACHED = {}


def _get_nc(t_total=T, tc=TC, n_cores=N_CORES):
    key = (t_total, tc, n_cores)
    if key not in _CACHED:
        t0 = time.time()
        _CACHED[key] = build_nc(t_total, tc, n_cores)
        _log(f"built bass program in {time.time() - t0:.1f}s")
    return _CACHED[key]


def make_in_maps(x, gate_w, gate_proj_w, up_proj_w, down_proj_w,
                 t_total=T, n_cores=N_CORES):
    x2d = np.ascontiguousarray(np.asarray(x, dtype=np.float32).reshape(t_total, D))
    xT = round_f32r(x2d.T)  # [D, T]
    gating = host_gating(x2d, np.asarray(gate_w, dtype=np.float32))  # [T, E]
    gating_r = round_f32r(gating.T)  # [E, T]
    in_maps = []
    for c in range(n_cores):
        in_maps.append({
            "xT": xT,
            "wgT": round_f32r(np.asarray(gate_proj_w[c], np.float32).T),
            "wuT": round_f32r(np.asarray(up_proj_w[c], np.float32).T),
            "wdT": round_f32r(np.asarray(down_proj_w[c], np.float32).T),
            "gcol": gating_r[c:c + 1, :],
            "ones": np.ones((1, P), dtype=np.float32),
        })
    return in_maps


def assemble_output(results, t_total=T, n_cores=N_CORES):
    shard = D // n_cores
    yT = np.empty((D, t_total), dtype=np.float32)
    for c in range(n_cores):
        yT[c * shard:(c + 1) * shard, :] = \
            results[c]["yshard"].reshape(shard, t_total)
    return np.ascontiguousarray(yT.T).reshape(B, S, D)


def _get_nc_routed(cap, tc=TC, n_cores=N_CORES):
    key = ("routed", cap, tc, n_cores)
    if key not in _CACHED:
        t0 = time.time()
        _CACHED[key] = build_nc_routed(cap, tc, n_cores)
        _log(f"built routed bass program (cap={cap}) in {time.time() - t0:.1f}s")
    return _CACHED[key]


def _round_up(v, m):
    return (v + m - 1) // m * m


def make_in_maps_routed(x, gate_w, gate_proj_w, up_proj_w, down_proj_w):
    """Returns (in_maps, idx_list, n_list, cap)."""
    from concurrent.futures import ThreadPoolExecutor

    x2d = np.ascontiguousarray(np.asarray(x, np.float32).reshape(T, D))
    x2d_r = round_f32r(x2d)
    gating = host_gating(x2d, np.asarray(gate_w, np.float32))  # [T, E]
    idx_list = [np.nonzero(gating[:, c] > 0)[0].astype(np.int64)
                for c in range(N_CORES)]
    n_list = [len(ix) for ix in idx_list]
    cap = _round_up(max(n_list), TC)

    ones = np.ones((1, P), dtype=np.float32)

    def prep_core(c):
        ix, n_c = idx_list[c], n_list[c]
        xcT = np.zeros((D, cap), dtype=np.float32)
        xcT[:, :n_c] = x2d_r[ix].T
        gprob = np.zeros((1, cap), dtype=np.float32)
        gprob[0, :n_c] = gating[ix, c]
        return {
            "xcT": xcT,
            "wgT": round_f32r(np.asarray(gate_proj_w[c], np.float32).T),
            "wuT": round_f32r(np.asarray(up_proj_w[c], np.float32).T),
            "wdT": round_f32r(np.asarray(down_proj_w[c], np.float32).T),
            "gprob": round_f32r(gprob),
            "ones": ones,
        }

    with ThreadPoolExecutor(N_CORES) as ex:
        in_maps = list(ex.map(prep_core, range(N_CORES)))
    return in_maps, idx_list, n_list, cap


def kernel(x, gate_w, gate_proj_w, up_proj_w, down_proj_w,
           num_experts_per_tok=2, _trace=False, _trace_cores=None):
    from concourse import bass_utils
    assert int(num_experts_per_tok) == TOPK
    mode = os.environ.get("KERNEL_MODE", "routed")

    kwargs = {}
    if _trace:
        try:
            sys.path.insert(0, os.path.dirname(os.path.abspath(__file__)))
            import axon_profile_shim
            axon_profile_shim.install()
        except Exception as exc:  # profiling is best-effort
            _log(f"profile shim unavailable: {exc}")
        kwargs = dict(trace=True,
                      trace_cores=_trace_cores or list(range(N_CORES)))

    if mode == "dense":
        nc = _get_nc()
        in_maps = make_in_maps(x, gate_w, gate_proj_w, up_proj_w, down_proj_w)
        t0 = time.time()
        res = bass_utils.run_bass_kernel_spmd(
            nc, in_maps, core_ids=list(range(N_CORES)), **kwargs)
        _log(f"run_bass_kernel_spmd took {time.time() - t0:.1f}s")
        kernel.last_result = res
        return assemble_output(res.results)

    # routed (default)
    t0 = time.time()
    in_maps, idx_list, n_list, cap = make_in_maps_routed(
        x, gate_w, gate_proj_w, up_proj_w, down_proj_w)
    _log(f"host prep {time.time() - t0:.1f}s (cap={cap}, counts={n_list})")
    nc = _get_nc_routed(cap)
    t0 = time.time()
    res = bass_utils.run_bass_kernel_spmd(
        nc, in_maps, core_ids=list(range(N_CORES)), **kwargs)
    _log(f"run_bass_kernel_spmd took {time.time() - t0:.1f}s")
    kernel.last_result = res
    t0 = time.time()
    y = np.zeros((T, D), dtype=np.float32)
    for c in range(N_CORES):
        yc = res.results[c]["ycomp"]  # [D, cap]
        y[idx_list[c]] += np.ascontiguousarray(yc[:, :n_list[c]].T)
    _log(f"host combine {time.time() - t0:.1f}s")
    return y.reshape(B, S, D)


kernel.last_result = None


# revision 12
# speedup vs baseline: 34.6829x; 1.1339x over previous
"""MoE feed-forward (8 experts, top-2, SwiGLU) on 8 Trainium2 NeuronCores.

Strategy: expert parallelism. Core c owns expert c and computes its expert's
SwiGLU output for all tokens with fp32r (FP22) matmuls, weights resident in
SBUF. Gating (router top-2 softmax) is computed on host in float64 and the
per-expert gating row is shipped as an input; each core scales its expert
output by its gating row, partial outputs are combined with an on-device
ReduceScatter, and the host reassembles the full output.
"""

import os
import sys
import time

sys.path.insert(0, "/opt/trn_rl_repo")

import numpy as np

# ---------------------------------------------------------------------------
# Problem constants (hardcoded per contract)
B, S, D, E, I, TOPK = 2, 2048, 1024, 8, 1408, 2
T = B * S  # 4096 tokens
P = 128
D_T = D // P   # 8 d-tiles
I_T = I // P   # 11 i-tiles
TC = 256       # token chunk (PSUM-bank free dim)
N_CORES = 8

_VERBOSE = bool(int(os.environ.get("KERNEL_VERBOSE", "0")))


def _log(msg):
    if _VERBOSE:
        print(f"[kernel] {msg}", flush=True)


def round_f32r(a: np.ndarray) -> np.ndarray:
    """RNE-round fp32 array to 13 mantissa bits (FP22 / e8m13)."""
    v = np.ascontiguousarray(a, dtype=np.float32).view(np.uint32)
    low = v & np.uint32(0x1FFF)
    base = v & np.uint32(0xFFFFE000)
    lsb = (v >> np.uint32(13)) & np.uint32(1)
    round_up = (low > np.uint32(0x1000)) | ((low == np.uint32(0x1000)) & (lsb == 1))
    out = base + (round_up.astype(np.uint32) << np.uint32(13))
    return out.view(np.float32)


def host_gating(x2d: np.ndarray, gate_w: np.ndarray):
    """Exact router: scores -> top-2 -> softmax. Returns gating [T, E] fp32."""
    scores = x2d.astype(np.float64) @ gate_w.astype(np.float64).T  # [T, E]
    idx = np.argsort(-scores, axis=-1, kind="stable")[:, :TOPK]  # [T, 2]
    top = np.take_along_axis(scores, idx, axis=-1)  # [T, 2] descending
    m = top[:, :1]
    ex = np.exp(top - m)
    probs = ex / ex.sum(axis=-1, keepdims=True)  # [T, 2]
    gating = np.zeros((x2d.shape[0], E), dtype=np.float64)
    np.put_along_axis(gating, idx, probs, axis=-1)
    return gating.astype(np.float32)


# ---------------------------------------------------------------------------
# Bass kernel builder


def build_nc(t_total=T, tc=TC, n_cores=N_CORES):
    import concourse.bass as bass  # noqa: F401
    import concourse.mybir as mybir
    import concourse.tile as tile
    from concourse import bacc

    f32 = mybir.dt.float32
    f32r = mybir.dt.float32r
    n_chunks = t_total // tc

    nc = bacc.Bacc("TRN2", debug=False, num_devices=n_cores)

    xT_d = nc.dram_tensor("xT", [D, t_total], f32r, kind="ExternalInput")
    wgT_d = nc.dram_tensor("wgT", [D, I], f32r, kind="ExternalInput")
    wuT_d = nc.dram_tensor("wuT", [D, I], f32r, kind="ExternalInput")
    wdT_d = nc.dram_tensor("wdT", [I, D], f32r, kind="ExternalInput")
    gcol_d = nc.dram_tensor("gcol", [1, t_total], f32r, kind="ExternalInput")
    ones_d = nc.dram_tensor("ones", [1, P], f32r, kind="ExternalInput")
    yshard_d = nc.dram_tensor("yshard", [D * t_total // n_cores], f32,
                              kind="ExternalOutput")

    xT_r = xT_d.ap().rearrange("(do dp) t -> dp do t", dp=P)
    wgT_r = wgT_d.ap().rearrange("(do dp) i -> dp do i", dp=P)
    wuT_r = wuT_d.ap().rearrange("(do dp) i -> dp do i", dp=P)
    wdT_r = wdT_d.ap().rearrange("(io ip) d -> ip io d", ip=P)

    with tile.TileContext(nc) as tc_ctx:
        tcx = tc_ctx
        with tcx.tile_pool(name="wpool", bufs=1) as wpool, \
             tcx.tile_pool(name="xpool", bufs=2) as xpool, \
             tcx.tile_pool(name="hpool", bufs=2) as hpool, \
             tcx.tile_pool(name="ypool", bufs=2) as ypool, \
             tcx.tile_pool(name="gspool", bufs=3) as gspool, \
             tcx.tile_pool(name="gbpool", bufs=2) as gbpool, \
             tcx.tile_pool(name="psg", bufs=2, space="PSUM") as psg, \
             tcx.tile_pool(name="psu", bufs=2, space="PSUM") as psu, \
             tcx.tile_pool(name="psy", bufs=2, space="PSUM") as psy, \
             tcx.tile_pool(name="psb", bufs=1, space="PSUM") as psb, \
             tcx.tile_pool(name="dram", bufs=1, space="DRAM") as dram:

            # ---- resident weights ----
            wg_sb = wpool.tile([P, D_T, I], f32r)
            wu_sb = wpool.tile([P, D_T, I], f32r)
            wd_sb = wpool.tile([P, I_T, D], f32r)
            for d_o in range(D_T):
                nc.sync.dma_start(wg_sb[:, d_o, :], wgT_r[:, d_o, :])
                nc.sync.dma_start(wu_sb[:, d_o, :], wuT_r[:, d_o, :])
            for i_o in range(I_T):
                nc.sync.dma_start(wd_sb[:, i_o, :], wdT_r[:, i_o, :])

            # gating row + ones column for partition-broadcast matmul
            gcol_sb = wpool.tile([1, t_total], f32r)
            nc.sync.dma_start(gcol_sb[:], gcol_d.ap())
            ones_sb = wpool.tile([1, P], f32r)
            nc.sync.dma_start(ones_sb[:], ones_d.ap())

            partial = dram.tile([D, t_total], f32)
            partial_r = partial.rearrange("(do dp) t -> dp do t", dp=P)
            rs_out = dram.tile([D * t_total // n_cores], f32)

            for ci in range(n_chunks):
                t0 = ci * tc
                xt = xpool.tile([P, D_T, tc], f32r, tag="xt")
                half = D_T // 2
                nc.sync.dma_start(xt[:, :half, :], xT_r[:, :half, t0:t0 + tc])
                nc.sync.dma_start(xt[:, half:, :], xT_r[:, half:, t0:t0 + tc])

                # broadcast gating row to 128 partitions for this chunk
                gb_ps = psb.tile([P, tc], f32, tag="gbps")
                nc.tensor.matmul(gb_ps[:], ones_sb[:], gcol_sb[:, t0:t0 + tc],
                                 start=True, stop=True)
                gb_sb = gbpool.tile([P, tc], f32, tag="gb")
                nc.scalar.copy(out=gb_sb[:], in_=gb_ps[:])

                h = hpool.tile([P, I_T, tc], f32r, tag="h")
                for i_o in range(I_T):
                    pg = psg.tile([P, tc], f32, tag="pg")
                    pu = psu.tile([P, tc], f32, tag="pu")
                    for d_o in range(D_T):
                        nc.tensor.matmul(
                            pg[:], wg_sb[:, d_o, i_o * P:(i_o + 1) * P],
                            xt[:, d_o, :],
                            start=(d_o == 0), stop=(d_o == D_T - 1))
                    for d_o in range(D_T):
                        nc.tensor.matmul(
                            pu[:], wu_sb[:, d_o, i_o * P:(i_o + 1) * P],
                            xt[:, d_o, :],
                            start=(d_o == 0), stop=(d_o == D_T - 1))
                    gs = gspool.tile([P, tc], f32r, tag="gs")
                    nc.scalar.activation(gs[:], pg[:],
                                         mybir.ActivationFunctionType.Silu)
                    nc.vector.tensor_mul(out=h[:, i_o, :], in0=gs[:], in1=pu[:])

                yout = ypool.tile([P, D_T, tc], f32, tag="yout")
                for d_o in range(D_T):
                    py = psy.tile([P, tc], f32, tag="py")
                    for i_o in range(I_T):
                        nc.tensor.matmul(
                            py[:], wd_sb[:, i_o, d_o * P:(d_o + 1) * P],
                            h[:, i_o, :],
                            start=(i_o == 0), stop=(i_o == I_T - 1))
                    nc.vector.tensor_mul(out=yout[:, d_o, :], in0=py[:],
                                         in1=gb_sb[:])
                nc.sync.dma_start(partial_r[:, :half, t0:t0 + tc],
                                  yout[:, :half, :])
                nc.sync.dma_start(partial_r[:, half:, t0:t0 + tc],
                                  yout[:, half:, :])

            nc.gpsimd.collective_compute(
                "ReduceScatter", mybir.AluOpType.add,
                replica_groups=[list(range(n_cores))],
                ins=[partial[:].opt()], outs=[rs_out[:].opt()])

            shard = D * t_total // n_cores
            q = shard // 4
            for k in range(4):
                nc.sync.dma_start(yshard_d.ap()[k * q:(k + 1) * q],
                                  rs_out[k * q:(k + 1) * q])

    nc.compile()
    return nc


def build_nc_routed(cap, tc=TC, n_cores=N_CORES):
    """Compact (routed) variant: each core computes its expert only for the
    `cap` tokens routed to it (host-gathered, feature-major). Output is the
    compact gated expert output, token-major [cap, D]; host scatter-adds."""
    import concourse.mybir as mybir
    import concourse.tile as tile
    from concourse import bacc

    f32 = mybir.dt.float32
    f32r = mybir.dt.float32r
    n_chunks = cap // tc
    assert cap % tc == 0

    nc = bacc.Bacc("TRN2", debug=False, num_devices=n_cores)

    xcT_d = nc.dram_tensor("xcT", [D, cap], f32r, kind="ExternalInput")
    wgT_d = nc.dram_tensor("wgT", [D, I], f32r, kind="ExternalInput")
    wuT_d = nc.dram_tensor("wuT", [D, I], f32r, kind="ExternalInput")
    wdT_d = nc.dram_tensor("wdT", [I, D], f32r, kind="ExternalInput")
    gprob_d = nc.dram_tensor("gprob", [1, cap], f32r, kind="ExternalInput")
    ones_d = nc.dram_tensor("ones", [1, P], f32r, kind="ExternalInput")
    ycomp_d = nc.dram_tensor("ycomp", [D, cap], f32, kind="ExternalOutput")

    xcT_r = xcT_d.ap().rearrange("(do dp) t -> dp do t", dp=P)
    wgT_r = wgT_d.ap().rearrange("(do dp) i -> dp do i", dp=P)
    wuT_r = wuT_d.ap().rearrange("(do dp) i -> dp do i", dp=P)
    wdT_r = wdT_d.ap().rearrange("(io ip) d -> ip io d", ip=P)
    # ycomp viewed as [dp, do, t]; D-major so each partition writes
    # contiguous `tc`-element runs (per-partition contiguity = DMA speed)
    ycomp_r = ycomp_d.ap().rearrange("(do dp) t -> dp do t", dp=P)

    with tile.TileContext(nc) as tcx:
        with tcx.tile_pool(name="wpool", bufs=1) as wpool, \
             tcx.tile_pool(name="xpool", bufs=2) as xpool, \
             tcx.tile_pool(name="hpool", bufs=2) as hpool, \
             tcx.tile_pool(name="ypool", bufs=2) as ypool, \
             tcx.tile_pool(name="gspool", bufs=3) as gspool, \
             tcx.tile_pool(name="gbpool", bufs=2) as gbpool, \
             tcx.tile_pool(name="psg", bufs=2, space="PSUM") as psg, \
             tcx.tile_pool(name="psu", bufs=2, space="PSUM") as psu, \
             tcx.tile_pool(name="psy", bufs=2, space="PSUM") as psy, \
             tcx.tile_pool(name="psb", bufs=1, space="PSUM") as psb:

            wg_sb = wpool.tile([P, D_T, I], f32r)
            wu_sb = wpool.tile([P, D_T, I], f32r)
            wd_sb = wpool.tile([P, I_T, D], f32r)
            half = D_T // 2
            ihalf = 6 * P  # i-tile boundary: first 6 i-tiles, then the rest

            def load_xt(ci, n_split=2):
                t0 = ci * tc
                xt = xpool.tile([P, D_T, tc], f32r, tag="xt", name=f"xt{ci}")
                step = D_T // n_split
                for s in range(0, D_T, step):
                    nc.sync.dma_start(xt[:, s:s + step, :],
                                      xcT_r[:, s:s + step, t0:t0 + tc])
                return xt

            # Issue order tuned for time-to-first-matmul: tiny tensors and
            # gate weights first (first-half i-tiles), then chunk-0
            # activations; up/down weights stream behind the first matmuls.
            gprob_sb = wpool.tile([1, cap], f32r)
            nc.sync.dma_start(gprob_sb[:], gprob_d.ap())
            ones_sb = wpool.tile([1, P], f32r)
            nc.sync.dma_start(ones_sb[:], ones_d.ap())
            for d_o in range(D_T):
                nc.sync.dma_start(wg_sb[:, d_o, :ihalf], wgT_r[:, d_o, :ihalf])
            xt_pre = {0: load_xt(0, n_split=4)}
            for d_o in range(D_T):
                nc.sync.dma_start(wu_sb[:, d_o, :ihalf], wuT_r[:, d_o, :ihalf])
            for d_o in range(D_T):
                nc.sync.dma_start(wg_sb[:, d_o, ihalf:], wgT_r[:, d_o, ihalf:])
            if n_chunks > 1:
                xt_pre[1] = load_xt(1)
            for d_o in range(D_T):
                nc.sync.dma_start(wu_sb[:, d_o, ihalf:], wuT_r[:, d_o, ihalf:])
            for i_o in range(I_T):
                nc.sync.dma_start(wd_sb[:, i_o, :], wdT_r[:, i_o, :])

            for ci in range(n_chunks):
                t0 = ci * tc
                xt = xt_pre[ci] if ci in xt_pre else load_xt(ci)

                gb_ps = psb.tile([P, tc], f32, tag="gbps")
                nc.tensor.matmul(gb_ps[:], ones_sb[:], gprob_sb[:, t0:t0 + tc],
                                 start=True, stop=True)
                gb_sb = gbpool.tile([P, tc], f32, tag="gb")
                nc.scalar.copy(out=gb_sb[:], in_=gb_ps[:])

                h = hpool.tile([P, I_T, tc], f32r, tag="h")
                for i_o in range(I_T):
                    pg = psg.tile([P, tc], f32, tag="pg")
                    pu = psu.tile([P, tc], f32, tag="pu")
                    for d_o in range(D_T):
                        nc.tensor.matmul(
                            pg[:], wg_sb[:, d_o, i_o * P:(i_o + 1) * P],
                            xt[:, d_o, :],
                            start=(d_o == 0), stop=(d_o == D_T - 1))
                    for d_o in range(D_T):
                        nc.tensor.matmul(
                            pu[:], wu_sb[:, d_o, i_o * P:(i_o + 1) * P],
                            xt[:, d_o, :],
                            start=(d_o == 0), stop=(d_o == D_T - 1))
                    gs = gspool.tile([P, tc], f32r, tag="gs")
                    nc.scalar.activation(gs[:], pg[:],
                                         mybir.ActivationFunctionType.Silu)
                    nc.vector.tensor_mul(out=h[:, i_o, :], in0=gs[:], in1=pu[:])

                yout = ypool.tile([P, D_T, tc], f32, tag="yout")
                for d_o in range(D_T):
                    py = psy.tile([P, tc], f32, tag="py")
                    for i_o in range(I_T):
                        nc.tensor.matmul(
                            py[:], wd_sb[:, i_o, d_o * P:(d_o + 1) * P],
                            h[:, i_o, :],
                            start=(i_o == 0), stop=(i_o == I_T - 1))
                    nc.vector.tensor_mul(out=yout[:, d_o, :], in0=py[:],
                                         in1=gb_sb[:])
                nc.sync.dma_start(ycomp_r[:, :half, t0:t0 + tc],
                                  yout[:, :half, :])
                nc.sync.dma_start(ycomp_r[:, half:, t0:t0 + tc],
                                  yout[:, half:, :])

    nc.compile()
    return nc


# ---------------------------------------------------------------------------
# Host-side wrapper

_CACHED = {}


def _get_nc(t_total=T, tc=TC, n_cores=N_CORES):
    key = (t_total, tc, n_cores)
    if key not in _CACHED:
        t0 = time.time()
        _CACHED[key] = build_nc(t_total, tc, n_cores)
        _log(f"built bass program in {time.time() - t0:.1f}s")
    return _CACHED[key]


def make_in_maps(x, gate_w, gate_proj_w, up_proj_w, down_proj_w,
                 t_total=T, n_cores=N_CORES):
    x2d = np.ascontiguousarray(np.asarray(x, dtype=np.float32).reshape(t_total, D))
    xT = round_f32r(x2d.T)  # [D, T]
    gating = host_gating(x2d, np.asarray(gate_w, dtype=np.float32))  # [T, E]
    gating_r = round_f32r(gating.T)  # [E, T]
    in_maps = []
    for c in range(n_cores):
        in_maps.append({
            "xT": xT,
            "wgT": round_f32r(np.asarray(gate_proj_w[c], np.float32).T),
            "wuT": round_f32r(np.asarray(up_proj_w[c], np.float32).T),
            "wdT": round_f32r(np.asarray(down_proj_w[c], np.float32).T),
            "gcol": gating_r[c:c + 1, :],
            "ones": np.ones((1, P), dtype=np.float32),
        })
    return in_maps


def assemble_output(results, t_total=T, n_cores=N_CORES):
    shard = D // n_cores
    yT = np.empty((D, t_total), dtype=np.float32)
    for c in range(n_cores):
        yT[c * shard:(c + 1) * shard, :] = \
            results[c]["yshard"].reshape(shard, t_total)
    return np.ascontiguousarray(yT.T).reshape(B, S, D)


def _get_nc_routed(cap, tc=TC, n_cores=N_CORES):
    key = ("routed", cap, tc, n_cores)
    if key not in _CACHED:
        t0 = time.time()
        _CACHED[key] = build_nc_routed(cap, tc, n_cores)
        _log(f"built routed bass program (cap={cap}) in {time.time() - t0:.1f}s")
    return _CACHED[key]


def _round_up(v, m):
    return (v + m - 1) // m * m


def make_in_maps_routed(x, gate_w, gate_proj_w, up_proj_w, down_proj_w):
    """Returns (in_maps, idx_list, n_list, cap, tc)."""
    from concurrent.futures import ThreadPoolExecutor

    x2d = np.ascontiguousarray(np.asarray(x, np.float32).reshape(T, D))
    x2d_r = round_f32r(x2d)
    gating = host_gating(x2d, np.asarray(gate_w, np.float32))  # [T, E]
    idx_list = [np.nonzero(gating[:, c] > 0)[0].astype(np.int64)
                for c in range(N_CORES)]
    n_list = [len(ix) for ix in idx_list]
    # Chunk width: PE issue is LDWEIGHTS-bound up to ~290 columns (~120 ns/MM
    # either way), so pick the fewest chunks whose width stays in [256, 320]:
    # fewer chunks = fewer matmul instructions at the same per-MM cost.
    max_n = max(n_list)
    n_chunks = max(1, -(-max_n // 320))
    tc = max(TC, _round_up(-(-max_n // n_chunks), 16))
    cap = tc * n_chunks

    ones = np.ones((1, P), dtype=np.float32)

    def prep_core(c):
        ix, n_c = idx_list[c], n_list[c]
        xcT = np.zeros((D, cap), dtype=np.float32)
        xcT[:, :n_c] = x2d_r[ix].T
        gprob = np.zeros((1, cap), dtype=np.float32)
        gprob[0, :n_c] = gating[ix, c]
        return {
            "xcT": xcT,
            "wgT": round_f32r(np.asarray(gate_proj_w[c], np.float32).T),
            "wuT": round_f32r(np.asarray(up_proj_w[c], np.float32).T),
            "wdT": round_f32r(np.asarray(down_proj_w[c], np.float32).T),
            "gprob": round_f32r(gprob),
            "ones": ones,
        }

    with ThreadPoolExecutor(N_CORES) as ex:
        in_maps = list(ex.map(prep_core, range(N_CORES)))
    return in_maps, idx_list, n_list, cap, tc


def kernel(x, gate_w, gate_proj_w, up_proj_w, down_proj_w,
           num_experts_per_tok=2, _trace=False, _trace_cores=None):
    from concourse import bass_utils
    assert int(num_experts_per_tok) == TOPK
    mode = os.environ.get("KERNEL_MODE", "routed")

    kwargs = {}
    if _trace:
        try:
            sys.path.insert(0, os.path.dirname(os.path.abspath(__file__)))
            import axon_profile_shim
            axon_profile_shim.install()
        except Exception as exc:  # profiling is best-effort
            _log(f"profile shim unavailable: {exc}")
        kwargs = dict(trace=True,
                      trace_cores=_trace_cores or list(range(N_CORES)))

    if mode == "dense":
        nc = _get_nc()
        in_maps = make_in_maps(x, gate_w, gate_proj_w, up_proj_w, down_proj_w)
        t0 = time.time()
        res = bass_utils.run_bass_kernel_spmd(
            nc, in_maps, core_ids=list(range(N_CORES)), **kwargs)
        _log(f"run_bass_kernel_spmd took {time.time() - t0:.1f}s")
        kernel.last_result = res
        return assemble_output(res.results)

    # routed (default)
    t0 = time.time()
    in_maps, idx_list, n_list, cap, tc = make_in_maps_routed(
        x, gate_w, gate_proj_w, up_proj_w, down_proj_w)
    _log(f"host prep {time.time() - t0:.1f}s (cap={cap}, tc={tc}, "
         f"counts={n_list})")
    nc = _get_nc_routed(cap, tc)
    t0 = time.time()
    res = bass_utils.run_bass_kernel_spmd(
        nc, in_maps, core_ids=list(range(N_CORES)), **kwargs)
    _log(f"run_bass_kernel_spmd took {time.time() - t0:.1f}s")
    kernel.last_result = res
    t0 = time.time()
    y = np.zeros((T, D), dtype=np.float32)
    for c in range(N_CORES):
        yc = res.results[c]["ycomp"]  # [D, cap]
        y[idx_list[c]] += np.ascontiguousarray(yc[:, :n_list[c]].T)
    _log(f"host combine {time.time() - t0:.1f}s")
    return y.reshape(B, S, D)


kernel.last_result = None


# revision 14
# speedup vs baseline: 35.5323x; 1.0245x over previous
"""MoE feed-forward (8 experts, top-2, SwiGLU) on 8 Trainium2 NeuronCores.

Strategy: expert parallelism. Core c owns expert c and computes its expert's
SwiGLU output for all tokens with fp32r (FP22) matmuls, weights resident in
SBUF. Gating (router top-2 softmax) is computed on host in float64 and the
per-expert gating row is shipped as an input; each core scales its expert
output by its gating row, partial outputs are combined with an on-device
ReduceScatter, and the host reassembles the full output.
"""

import os
import sys
import time

sys.path.insert(0, "/opt/trn_rl_repo")

import numpy as np

# ---------------------------------------------------------------------------
# Problem constants (hardcoded per contract)
B, S, D, E, I, TOPK = 2, 2048, 1024, 8, 1408, 2
T = B * S  # 4096 tokens
P = 128
D_T = D // P   # 8 d-tiles
I_T = I // P   # 11 i-tiles
TC = 256       # token chunk (PSUM-bank free dim)
N_CORES = 8

_VERBOSE = bool(int(os.environ.get("KERNEL_VERBOSE", "0")))


def _log(msg):
    if _VERBOSE:
        print(f"[kernel] {msg}", flush=True)


def round_f32r(a: np.ndarray) -> np.ndarray:
    """RNE-round fp32 array to 13 mantissa bits (FP22 / e8m13)."""
    v = np.ascontiguousarray(a, dtype=np.float32).view(np.uint32)
    low = v & np.uint32(0x1FFF)
    base = v & np.uint32(0xFFFFE000)
    lsb = (v >> np.uint32(13)) & np.uint32(1)
    round_up = (low > np.uint32(0x1000)) | ((low == np.uint32(0x1000)) & (lsb == 1))
    out = base + (round_up.astype(np.uint32) << np.uint32(13))
    return out.view(np.float32)


def host_gating(x2d: np.ndarray, gate_w: np.ndarray):
    """Exact router: scores -> top-2 -> softmax. Returns gating [T, E] fp32."""
    scores = x2d.astype(np.float64) @ gate_w.astype(np.float64).T  # [T, E]
    idx = np.argsort(-scores, axis=-1, kind="stable")[:, :TOPK]  # [T, 2]
    top = np.take_along_axis(scores, idx, axis=-1)  # [T, 2] descending
    m = top[:, :1]
    ex = np.exp(top - m)
    probs = ex / ex.sum(axis=-1, keepdims=True)  # [T, 2]
    gating = np.zeros((x2d.shape[0], E), dtype=np.float64)
    np.put_along_axis(gating, idx, probs, axis=-1)
    return gating.astype(np.float32)


# ---------------------------------------------------------------------------
# Bass kernel builder


def build_nc(t_total=T, tc=TC, n_cores=N_CORES):
    import concourse.bass as bass  # noqa: F401
    import concourse.mybir as mybir
    import concourse.tile as tile
    from concourse import bacc

    f32 = mybir.dt.float32
    f32r = mybir.dt.float32r
    n_chunks = t_total // tc

    nc = bacc.Bacc("TRN2", debug=False, num_devices=n_cores)

    xT_d = nc.dram_tensor("xT", [D, t_total], f32r, kind="ExternalInput")
    wgT_d = nc.dram_tensor("wgT", [D, I], f32r, kind="ExternalInput")
    wuT_d = nc.dram_tensor("wuT", [D, I], f32r, kind="ExternalInput")
    wdT_d = nc.dram_tensor("wdT", [I, D], f32r, kind="ExternalInput")
    gcol_d = nc.dram_tensor("gcol", [1, t_total], f32r, kind="ExternalInput")
    ones_d = nc.dram_tensor("ones", [1, P], f32r, kind="ExternalInput")
    yshard_d = nc.dram_tensor("yshard", [D * t_total // n_cores], f32,
                              kind="ExternalOutput")

    xT_r = xT_d.ap().rearrange("(do dp) t -> dp do t", dp=P)
    wgT_r = wgT_d.ap().rearrange("(do dp) i -> dp do i", dp=P)
    wuT_r = wuT_d.ap().rearrange("(do dp) i -> dp do i", dp=P)
    wdT_r = wdT_d.ap().rearrange("(io ip) d -> ip io d", ip=P)

    with tile.TileContext(nc) as tc_ctx:
        tcx = tc_ctx
        with tcx.tile_pool(name="wpool", bufs=1) as wpool, \
             tcx.tile_pool(name="xpool", bufs=2) as xpool, \
             tcx.tile_pool(name="hpool", bufs=2) as hpool, \
             tcx.tile_pool(name="ypool", bufs=2) as ypool, \
             tcx.tile_pool(name="gspool", bufs=3) as gspool, \
             tcx.tile_pool(name="gbpool", bufs=2) as gbpool, \
             tcx.tile_pool(name="psg", bufs=2, space="PSUM") as psg, \
             tcx.tile_pool(name="psu", bufs=2, space="PSUM") as psu, \
             tcx.tile_pool(name="psy", bufs=2, space="PSUM") as psy, \
             tcx.tile_pool(name="psb", bufs=1, space="PSUM") as psb, \
             tcx.tile_pool(name="dram", bufs=1, space="DRAM") as dram:

            # ---- resident weights ----
            wg_sb = wpool.tile([P, D_T, I], f32r)
            wu_sb = wpool.tile([P, D_T, I], f32r)
            wd_sb = wpool.tile([P, I_T, D], f32r)
            for d_o in range(D_T):
                nc.sync.dma_start(wg_sb[:, d_o, :], wgT_r[:, d_o, :])
                nc.sync.dma_start(wu_sb[:, d_o, :], wuT_r[:, d_o, :])
            for i_o in range(I_T):
                nc.sync.dma_start(wd_sb[:, i_o, :], wdT_r[:, i_o, :])

            # gating row + ones column for partition-broadcast matmul
            gcol_sb = wpool.tile([1, t_total], f32r)
            nc.sync.dma_start(gcol_sb[:], gcol_d.ap())
            ones_sb = wpool.tile([1, P], f32r)
            nc.sync.dma_start(ones_sb[:], ones_d.ap())

            partial = dram.tile([D, t_total], f32)
            partial_r = partial.rearrange("(do dp) t -> dp do t", dp=P)
            rs_out = dram.tile([D * t_total // n_cores], f32)

            for ci in range(n_chunks):
                t0 = ci * tc
                xt = xpool.tile([P, D_T, tc], f32r, tag="xt")
                half = D_T // 2
                nc.sync.dma_start(xt[:, :half, :], xT_r[:, :half, t0:t0 + tc])
                nc.sync.dma_start(xt[:, half:, :], xT_r[:, half:, t0:t0 + tc])

                # broadcast gating row to 128 partitions for this chunk
                gb_ps = psb.tile([P, tc], f32, tag="gbps")
                nc.tensor.matmul(gb_ps[:], ones_sb[:], gcol_sb[:, t0:t0 + tc],
                                 start=True, stop=True)
                gb_sb = gbpool.tile([P, tc], f32, tag="gb")
                nc.scalar.copy(out=gb_sb[:], in_=gb_ps[:])

                h = hpool.tile([P, I_T, tc], f32r, tag="h")
                for i_o in range(I_T):
                    pg = psg.tile([P, tc], f32, tag="pg")
                    pu = psu.tile([P, tc], f32, tag="pu")
                    for d_o in range(D_T):
                        nc.tensor.matmul(
                            pg[:], wg_sb[:, d_o, i_o * P:(i_o + 1) * P],
                            xt[:, d_o, :],
                            start=(d_o == 0), stop=(d_o == D_T - 1))
                    for d_o in range(D_T):
                        nc.tensor.matmul(
                            pu[:], wu_sb[:, d_o, i_o * P:(i_o + 1) * P],
                            xt[:, d_o, :],
                            start=(d_o == 0), stop=(d_o == D_T - 1))
                    gs = gspool.tile([P, tc], f32r, tag="gs")
                    nc.scalar.activation(gs[:], pg[:],
                                         mybir.ActivationFunctionType.Silu)
                    nc.vector.tensor_mul(out=h[:, i_o, :], in0=gs[:], in1=pu[:])

                yout = ypool.tile([P, D_T, tc], f32, tag="yout")
                for d_o in range(D_T):
                    py = psy.tile([P, tc], f32, tag="py")
                    for i_o in range(I_T):
                        nc.tensor.matmul(
                            py[:], wd_sb[:, i_o, d_o * P:(d_o + 1) * P],
                            h[:, i_o, :],
                            start=(i_o == 0), stop=(i_o == I_T - 1))
                    nc.vector.tensor_mul(out=yout[:, d_o, :], in0=py[:],
                                         in1=gb_sb[:])
                nc.sync.dma_start(partial_r[:, :half, t0:t0 + tc],
                                  yout[:, :half, :])
                nc.sync.dma_start(partial_r[:, half:, t0:t0 + tc],
                                  yout[:, half:, :])

            nc.gpsimd.collective_compute(
                "ReduceScatter", mybir.AluOpType.add,
                replica_groups=[list(range(n_cores))],
                ins=[partial[:].opt()], outs=[rs_out[:].opt()])

            shard = D * t_total // n_cores
            q = shard // 4
            for k in range(4):
                nc.sync.dma_start(yshard_d.ap()[k * q:(k + 1) * q],
                                  rs_out[k * q:(k + 1) * q])

    nc.compile()
    return nc


def build_nc_routed(cap, tc=TC, n_cores=N_CORES):
    """Compact (routed) variant: each core computes its expert only for the
    `cap` tokens routed to it (host-gathered, feature-major). Output is the
    compact gated expert output, token-major [cap, D]; host scatter-adds."""
    import concourse.mybir as mybir
    import concourse.tile as tile
    from concourse import bacc

    f32 = mybir.dt.float32
    f32r = mybir.dt.float32r
    n_chunks = cap // tc
    assert cap % tc == 0

    nc = bacc.Bacc("TRN2", debug=False, num_devices=n_cores)

    xcT_d = nc.dram_tensor("xcT", [D, cap], f32r, kind="ExternalInput")
    wgT_d = nc.dram_tensor("wgT", [D, I], f32r, kind="ExternalInput")
    wuT_d = nc.dram_tensor("wuT", [D, I], f32r, kind="ExternalInput")
    wdT_d = nc.dram_tensor("wdT", [I, D], f32r, kind="ExternalInput")
    gprob_d = nc.dram_tensor("gprob", [1, cap], f32r, kind="ExternalInput")
    ones_d = nc.dram_tensor("ones", [1, P], f32r, kind="ExternalInput")
    ycomp_d = nc.dram_tensor("ycomp", [D, cap], f32, kind="ExternalOutput")

    xcT_r = xcT_d.ap().rearrange("(do dp) t -> dp do t", dp=P)
    wgT_r = wgT_d.ap().rearrange("(do dp) i -> dp do i", dp=P)
    wuT_r = wuT_d.ap().rearrange("(do dp) i -> dp do i", dp=P)
    wdT_r = wdT_d.ap().rearrange("(io ip) d -> ip io d", ip=P)
    # ycomp viewed as [dp, do, t]; D-major so each partition writes
    # contiguous `tc`-element runs (per-partition contiguity = DMA speed)
    ycomp_r = ycomp_d.ap().rearrange("(do dp) t -> dp do t", dp=P)

    with tile.TileContext(nc) as tcx:
        with tcx.tile_pool(name="wpool", bufs=1) as wpool, \
             tcx.tile_pool(name="xpool", bufs=2) as xpool, \
             tcx.tile_pool(name="hpool", bufs=2) as hpool, \
             tcx.tile_pool(name="ypool", bufs=2) as ypool, \
             tcx.tile_pool(name="gspool", bufs=3) as gspool, \
             tcx.tile_pool(name="gbpool", bufs=2) as gbpool, \
             tcx.tile_pool(name="psg", bufs=2, space="PSUM") as psg, \
             tcx.tile_pool(name="psu", bufs=2, space="PSUM") as psu, \
             tcx.tile_pool(name="psy", bufs=2, space="PSUM") as psy, \
             tcx.tile_pool(name="psb", bufs=1, space="PSUM") as psb:

            wg_sb = wpool.tile([P, D_T, I], f32r)
            wu_sb = wpool.tile([P, D_T, I], f32r)
            wd_sb = wpool.tile([P, I_T, D], f32r)
            half = D_T // 2
            ihalf = 6 * P  # i-tile boundary: first 6 i-tiles, then the rest

            def load_xt(ci, n_split=2):
                t0 = ci * tc
                xt = xpool.tile([P, D_T, tc], f32r, tag="xt", name=f"xt{ci}")
                step = D_T // n_split
                for s in range(0, D_T, step):
                    nc.sync.dma_start(xt[:, s:s + step, :],
                                      xcT_r[:, s:s + step, t0:t0 + tc])
                return xt

            # Issue order tuned for time-to-first-matmul: tiny tensors and
            # gate weights first (first-half i-tiles), then chunk-0
            # activations; up/down weights stream behind the first matmuls.
            gprob_sb = wpool.tile([1, cap], f32r)
            nc.sync.dma_start(gprob_sb[:], gprob_d.ap())
            ones_sb = wpool.tile([1, P], f32r)
            nc.sync.dma_start(ones_sb[:], ones_d.ap())
            for d_o in range(D_T):
                nc.sync.dma_start(wg_sb[:, d_o, :ihalf], wgT_r[:, d_o, :ihalf])
            xt_pre = {0: load_xt(0, n_split=4)}
            for d_o in range(D_T):
                nc.sync.dma_start(wu_sb[:, d_o, :ihalf], wuT_r[:, d_o, :ihalf])
            for d_o in range(D_T):
                nc.sync.dma_start(wg_sb[:, d_o, ihalf:], wgT_r[:, d_o, ihalf:])
            if n_chunks > 1:
                xt_pre[1] = load_xt(1)
            for d_o in range(D_T):
                nc.sync.dma_start(wu_sb[:, d_o, ihalf:], wuT_r[:, d_o, ihalf:])
            for i_o in range(I_T):
                nc.sync.dma_start(wd_sb[:, i_o, :], wdT_r[:, i_o, :])

            for ci in range(n_chunks):
                t0 = ci * tc
                xt = xt_pre[ci] if ci in xt_pre else load_xt(ci)

                gb_ps = psb.tile([P, tc], f32, tag="gbps")
                nc.tensor.matmul(gb_ps[:], ones_sb[:], gprob_sb[:, t0:t0 + tc],
                                 start=True, stop=True)
                gb_sb = gbpool.tile([P, tc], f32, tag="gb")
                nc.scalar.copy(out=gb_sb[:], in_=gb_ps[:])

                h = hpool.tile([P, I_T, tc], f32r, tag="h")
                for i_o in range(I_T):
                    pg = psg.tile([P, tc], f32, tag="pg")
                    pu = psu.tile([P, tc], f32, tag="pu")
                    for d_o in range(D_T):
                        nc.tensor.matmul(
                            pg[:], wg_sb[:, d_o, i_o * P:(i_o + 1) * P],
                            xt[:, d_o, :],
                            start=(d_o == 0), stop=(d_o == D_T - 1))
                    for d_o in range(D_T):
                        nc.tensor.matmul(
                            pu[:], wu_sb[:, d_o, i_o * P:(i_o + 1) * P],
                            xt[:, d_o, :],
                            start=(d_o == 0), stop=(d_o == D_T - 1))
                    gs = gspool.tile([P, tc], f32r, tag="gs")
                    nc.scalar.activation(gs[:], pg[:],
                                         mybir.ActivationFunctionType.Silu)
                    nc.vector.tensor_mul(out=h[:, i_o, :], in0=gs[:], in1=pu[:])

                yout = ypool.tile([P, D_T, tc], f32, tag="yout")
                for d_o in range(D_T):
                    py = psy.tile([P, tc], f32, tag="py")
                    for i_o in range(I_T):
                        nc.tensor.matmul(
                            py[:], wd_sb[:, i_o, d_o * P:(d_o + 1) * P],
                            h[:, i_o, :],
                            start=(i_o == 0), stop=(i_o == I_T - 1))
                    nc.vector.tensor_mul(out=yout[:, d_o, :], in0=py[:],
                                         in1=gb_sb[:])
                nc.sync.dma_start(ycomp_r[:, :half, t0:t0 + tc],
                                  yout[:, :half, :])
                nc.sync.dma_start(ycomp_r[:, half:, t0:t0 + tc],
                                  yout[:, half:, :])

    nc.compile()
    return nc


# ---------------------------------------------------------------------------
# Host-side wrapper

_CACHED = {}


def _get_nc(t_total=T, tc=TC, n_cores=N_CORES):
    key = (t_total, tc, n_cores)
    if key not in _CACHED:
        t0 = time.time()
        _CACHED[key] = build_nc(t_total, tc, n_cores)
        _log(f"built bass program in {time.time() - t0:.1f}s")
    return _CACHED[key]


def make_in_maps(x, gate_w, gate_proj_w, up_proj_w, down_proj_w,
                 t_total=T, n_cores=N_CORES):
    x2d = np.ascontiguousarray(np.asarray(x, dtype=np.float32).reshape(t_total, D))
    xT = round_f32r(x2d.T)  # [D, T]
    gating = host_gating(x2d, np.asarray(gate_w, dtype=np.float32))  # [T, E]
    gating_r = round_f32r(gating.T)  # [E, T]
    in_maps = []
    for c in range(n_cores):
        in_maps.append({
            "xT": xT,
            "wgT": round_f32r(np.asarray(gate_proj_w[c], np.float32).T),
            "wuT": round_f32r(np.asarray(up_proj_w[c], np.float32).T),
            "wdT": round_f32r(np.asarray(down_proj_w[c], np.float32).T),
            "gcol": gating_r[c:c + 1, :],
            "ones": np.ones((1, P), dtype=np.float32),
        })
    return in_maps


def assemble_output(results, t_total=T, n_cores=N_CORES):
    shard = D // n_cores
    yT = np.empty((D, t_total), dtype=np.float32)
    for c in range(n_cores):
        yT[c * shard:(c + 1) * shard, :] = \
            results[c]["yshard"].reshape(shard, t_total)
    return np.ascontiguousarray(yT.T).reshape(B, S, D)


def _get_nc_routed(cap, tc=TC, n_cores=N_CORES):
    key = ("routed", cap, tc, n_cores)
    if key not in _CACHED:
        t0 = time.time()
        _CACHED[key] = build_nc_routed(cap, tc, n_cores)
        _log(f"built routed bass program (cap={cap}) in {time.time() - t0:.1f}s")
    return _CACHED[key]


def _round_up(v, m):
    return (v + m - 1) // m * m


def make_in_maps_routed(x, gate_w, gate_proj_w, up_proj_w, down_proj_w):
    """Returns (in_maps, idx_list, n_list, cap, tc)."""
    from concurrent.futures import ThreadPoolExecutor

    x2d = np.ascontiguousarray(np.asarray(x, np.float32).reshape(T, D))
    x2d_r = round_f32r(x2d)
    gating = host_gating(x2d, np.asarray(gate_w, np.float32))  # [T, E]
    idx_list = [np.nonzero(gating[:, c] > 0)[0].astype(np.int64)
                for c in range(N_CORES)]
    n_list = [len(ix) for ix in idx_list]
    # Chunk width: PE issue is LDWEIGHTS-bound up to ~290 columns (~120 ns/MM
    # either way), so pick the fewest chunks whose width stays in [256, 320]:
    # fewer chunks = fewer matmul instructions at the same per-MM cost.
    max_n = max(n_list)
    n_chunks = max(1, -(-max_n // 320))
    tc = max(TC, _round_up(-(-max_n // n_chunks), 16))
    cap = tc * n_chunks

    ones = np.ones((1, P), dtype=np.float32)

    def prep_core(c):
        ix, n_c = idx_list[c], n_list[c]
        xcT = np.zeros((D, cap), dtype=np.float32)
        xcT[:, :n_c] = x2d_r[ix].T
        gprob = np.zeros((1, cap), dtype=np.float32)
        gprob[0, :n_c] = gating[ix, c]
        return {
            "xcT": xcT,
            "wgT": round_f32r(np.asarray(gate_proj_w[c], np.float32).T),
            "wuT": round_f32r(np.asarray(up_proj_w[c], np.float32).T),
            "wdT": round_f32r(np.asarray(down_proj_w[c], np.float32).T),
            "gprob": round_f32r(gprob),
            "ones": ones,
        }

    with ThreadPoolExecutor(N_CORES) as ex:
        in_maps = list(ex.map(prep_core, range(N_CORES)))
    return in_maps, idx_list, n_list, cap, tc


def kernel(x, gate_w, gate_proj_w, up_proj_w, down_proj_w,
           num_experts_per_tok=2, _trace=False, _trace_cores=None):
    from concourse import bass_utils
    assert int(num_experts_per_tok) == TOPK
    mode = os.environ.get("KERNEL_MODE", "routed")

    kwargs = {}
    if _trace:
        try:
            sys.path.insert(0, os.path.dirname(os.path.abspath(__file__)))
            import axon_profile_shim
            axon_profile_shim.install()
        except Exception as exc:  # profiling is best-effort
            _log(f"profile shim unavailable: {exc}")
        kwargs = dict(trace=True,
                      trace_cores=_trace_cores or list(range(N_CORES)))

    if mode == "dense":
        nc = _get_nc()
        in_maps = make_in_maps(x, gate_w, gate_proj_w, up_proj_w, down_proj_w)
        t0 = time.time()
        res = bass_utils.run_bass_kernel_spmd(
            nc, in_maps, core_ids=list(range(N_CORES)), **kwargs)
        _log(f"run_bass_kernel_spmd took {time.time() - t0:.1f}s")
        kernel.last_result = res
        return assemble_output(res.results)

    # routed (default)
    t0 = time.time()
    in_maps, idx_list, n_list, cap, tc = make_in_maps_routed(
        x, gate_w, gate_proj_w, up_proj_w, down_proj_w)
    _log(f"host prep {time.time() - t0:.1f}s (cap={cap}, tc={tc}, "
         f"counts={n_list})")
    nc = _get_nc_routed(cap, tc)
    t0 = time.time()
    res = bass_utils.run_bass_kernel_spmd(
        nc, in_maps, core_ids=list(range(N_CORES)), **kwargs)
    _log(f"run_bass_kernel_spmd took {time.time() - t0:.1f}s")
    kernel.last_result = res
    t0 = time.time()
    y = np.zeros((T, D), dtype=np.float32)
    for c in range(N_CORES):
        yc = res.results[c]["ycomp"]  # [D, cap]
        y[idx_list[c]] += np.ascontiguousarray(yc[:, :n_list[c]].T)
    _log(f"host combine {time.time() - t0:.1f}s")
    return y.reshape(B, S, D)


kernel.last_result = None
